# revision 3
# baseline (speedup 1.0000x reference)
"""GAT (3-layer, PyG-style) Trainium2 Bass kernel, 8-core dst-sharded.

Self-contained: takes full inputs, shards internally, returns full output.

Design:
  - dst nodes sharded across 8 cores (graph parallel per the sharding hint).
  - Per layer (3 SPMD launches; host only shards/permutes/transposes/casts
    between them):
    dense phase: node-major DRAM gather-table rows [hW*bn_s (bf16) | al_s f32]
      built by PE matmuls (lhsT = h^T chunks, rhs = W_aug), plus dense al_d.
    edge phase: padded-CSR slots (K slots per dst per src-half, K bucketed by
      max src-half degree), dma_gather of one 256B/512B table row per slot,
      ex = exp(leaky_relu(al_s + al_d)), messages scaled by ex, segment-sum
      via fixed shifted selection-matrix matmuls accumulating
      [msg_sums | sum_ex] per 128-dst window in PSUM; drain divides by
      sum_ex and adds skip matmul + bias (+BN fold, +ReLU).
  - src space is split in two halves with separate table bases so dma_gather's
    int16 indices stay < 32768; padded slots gather a sentinel row whose
    al_s = -40 (=> ex ~ 0) and whose message contribution ~ 0.
"""
import numpy as np
import ml_dtypes

import concourse.bacc as bacc
import concourse.mybir as mybir
import concourse.tile as tile
from concourse.alu_op_type import AluOpType
from concourse.bass_utils import run_bass_kernel_spmd

BF16 = mybir.dt.bfloat16
F32 = mybir.dt.float32
I16 = mybir.dt.int16

NC = 8
KLIST = (8, 16, 32, 64, 128)
P = 128
EPS = 1e-5
SENT_ALS = -40.0


def _round_up(x, m):
    return (x + m - 1) // m * m


# ----------------------------------------------------------------- planning

def build_plan(src, dst, N):
    D = N // NC
    HALF = N // 2
    core = dst // D
    dloc = dst % D
    half = (src >= HALF).astype(np.int64)

    deg = np.zeros((NC, D, 2), np.int64)
    np.add.at(deg, (core, dloc, half), 1)
    mx = deg.max(axis=2)  # [NC, D]
    Kd = np.select([mx <= 8, mx <= 16, mx <= 32, mx <= 64], [8, 16, 32, 64], 128)
    assert mx.max() <= 128, f"degree bucket overflow: {mx.max()}"

    nK = {k: _round_up(int((Kd == k).sum(axis=1).max()), 16) for k in KLIST}
    Dp = sum(nK.values())
    nK[8] += (-Dp) % 128
    Dp = sum(nK.values())
    off = {}
    o = 0
    for k in KLIST:
        off[k] = o
        o += nK[k]

    slabs = []
    for k in KLIST:
        q = P // k
        for i in range(nK[k] * k // P):
            slabs.append((k, off[k] + i * q))
    nslab = len(slabs)

    slab_win = [vd0 // P for (_, vd0) in slabs]
    first_slab = {}
    last_slab = {}
    for i, w in enumerate(slab_win):
        first_slab.setdefault(w, i)
        last_slab[w] = i
    nwin = Dp // P

    slot0 = np.zeros(Dp, np.int64)
    for si, (k, vd0) in enumerate(slabs):
        q = P // k
        for j in range(q):
            slot0[vd0 + j] = si * P + j * k
    TOT = _round_up(nslab, 64) * P

    shared = dict(N=N, D=D, HALF=HALF, Dp=Dp, nK=nK, off=off, slabs=slabs,
                  slab_win=slab_win, first_slab=first_slab,
                  last_slab=last_slab, nwin=nwin, TOT=TOT)

    plans = []
    for c in range(NC):
        vid = np.full(D, -1, np.int64)
        vmap = np.full(Dp, -1, np.int64)
        used = {k: 0 for k in KLIST}
        order = np.argsort(Kd[c], kind="stable")
        for d in order:
            k = int(Kd[c, d])
            pos = off[k] + used[k]
            used[k] += 1
            vid[d] = pos
            vmap[pos] = d
        em = core == c
        es = src[em]
        evd = vid[dloc[em]]
        eh = half[em]
        key = evd * 2 + eh
        si = np.argsort(key, kind="stable")
        ks = key[si]
        starts = np.zeros(2 * Dp + 1, np.int64)
        np.cumsum(np.bincount(ks, minlength=2 * Dp), out=starts[1:])
        rank = np.arange(len(ks)) - starts[ks]
        spos = slot0[evd[si]] + rank
        essorted = es[si]
        lo = np.full(TOT, HALF, np.int64)
        hi = np.full(TOT, HALF, np.int64)
        mlo = ks % 2 == 0
        lo[spos[mlo]] = essorted[mlo]
        hi[spos[~mlo]] = essorted[~mlo] - HALF
        plans.append(dict(vmap=vmap, idx_lo=_wrap16(lo), idx_hi=_wrap16(hi)))
    return shared, plans


def _wrap16(stream):
    TOT = len(stream)
    w = stream.reshape(TOT // 16, 16).T.astype(np.int16)
    return np.tile(w, (8, 1))


def _svar_layout():
    """index map + host array of shifted selection matrices."""
    idx = {}
    mats = []
    for k in KLIST:
        q = P // k
        for s in range(P // q):
            idx[(k, s)] = len(mats)
            m = np.zeros((P, P), np.float32)
            for p in range(P):
                m[p, s * q + p // k] = 1.0
            mats.append(m)
    return idx, np.concatenate(mats, 0)


SVAR_INDEX, SVAR_NP = _svar_layout()
NSV = len(SVAR_INDEX)


# ------------------------------------------------------------- kernel build

def build_layer(shared, F, OUTW, relu, mean_heads, tcap, gb, dbg_stage=99):
    N, HALF, Dp = shared["N"], shared["HALF"], shared["Dp"]
    nwin = shared["nwin"]
    slabs = shared["slabs"]
    slab_win = shared["slab_win"]
    first_slab, last_slab = shared["first_slab"], shared["last_slab"]
    TOT = shared["TOT"]
    nslab = len(slabs)
    assert 64 % tcap == 0 or tcap % 64 == 0
    groups = []
    s0 = 0
    while s0 < nslab:
        groups.append((s0, min(s0 + tcap, nslab)))
        s0 += tcap

    RW = 128 if OUTW == 64 else 256
    NA = 72 if OUTW == 64 else 260
    NAW = OUTW + 4
    PADW = 128 if OUTW == 64 else 512
    Npad = _round_up(N, P)
    nchunk = Npad // P
    TROWS = Npad + 2

    nc = bacc.Bacc("TRN2", target_bir_lowering=False, debug=False)
    hT = nc.dram_tensor("hT", [F, Npad], F32, kind="ExternalInput")
    hTow = nc.dram_tensor("hTow", [F, Dp], F32, kind="ExternalInput")
    Waug = nc.dram_tensor("Waug", [F, NA], F32, kind="ExternalInput")
    skipW = nc.dram_tensor("skipW", [F, 64], F32, kind="ExternalInput")
    biasR = nc.dram_tensor("biasR", [P, 64], F32, kind="ExternalInput")
    svar_in = nc.dram_tensor("svar", [NSV * P, P], BF16, kind="ExternalInput")
    rep_in = nc.dram_tensor("rep", [len(KLIST) * 16, P], F32, kind="ExternalInput")
    sent_in = nc.dram_tensor("sent", [2, RW], BF16, kind="ExternalInput")
    idx_lo = nc.dram_tensor("idx_lo", [P, TOT // 16], I16, kind="ExternalInput")
    idx_hi = nc.dram_tensor("idx_hi", [P, TOT // 16], I16, kind="ExternalInput")
    a2s_in = (nc.dram_tensor("a2s", [P, 1, 256], BF16, kind="ExternalInput")
              if mean_heads else None)

    table = nc.dram_tensor("table", [TROWS, RW], BF16, kind="Internal")
    aldv_d = nc.dram_tensor("aldv", [Dp, 4], F32, kind="Internal")
    y_out = nc.dram_tensor("y", [Dp, 64], F32, kind="ExternalOutput")

    def table_row_ranges(n0, n1):
        out = []
        cuts = sorted({n0, min(max(HALF, n0), n1), n1})
        for a, b in zip(cuts, cuts[1:]):
            if a >= b:
                continue
            out.append((a, b, a if a < HALF else a + 1))
        return out

    with tile.TileContext(nc) as tc:
        with (
            tc.tile_pool(name="const", bufs=1) as cp,
            tc.tile_pool(name="ybuf", bufs=1) as yp,
        ):
            waug_sb = cp.tile([F, NA], F32)
            nc.sync.dma_start(waug_sb[:], Waug[:])
            skipw_sb = cp.tile([F, 64], F32)
            nc.sync.dma_start(skipw_sb[:], skipW[:])
            bias_sb = cp.tile([P, 64], F32)
            nc.sync.dma_start(bias_sb[:], biasR[:])
            svar_sb = cp.tile([P, NSV, P], BF16)
            nc.sync.dma_start(svar_sb[:],
                              svar_in[:].rearrange("(v p) c -> p v c", p=P))
            rep_sb = cp.tile([16, len(KLIST), P], F32)
            nc.sync.dma_start(rep_sb[:],
                              rep_in[:].rearrange("(v p) c -> p v c", p=16))
            hTow_sb = cp.tile([F, Dp], F32)
            nc.sync.dma_start(hTow_sb[:], hTow[:])
            if mean_heads:
                a2s_sb = cp.tile([P, 1, 256], BF16)
                nc.sync.dma_start(a2s_sb[:], a2s_in[:])
            y_sb = yp.tile([P, nwin, 64], F32)
            nc.gpsimd.memset(y_sb[:], 0)

            # ---------------- dense phase: gather table + dense al_d
            with (
                tc.tile_pool(name="dstage", bufs=3) as dsp,
                tc.tile_pool(name="pdense", bufs=2, space="PSUM") as pd,
                tc.tile_pool(name="pal", bufs=1, space="PSUM") as pal,
            ):
                for g0 in range(0, nchunk, gb):
                    g1 = min(g0 + gb, nchunk)
                    ng = g1 - g0
                    stage = dsp.tile([F, gb * P], F32, tag="stage")
                    nc.sync.dma_start(stage[:, : ng * P], hT[:, g0 * P:g1 * P])
                    dps = pd.tile([P, gb * PADW], F32, space="PSUM", tag="dps")
                    for i in range(ng):
                        nc.tensor.matmul(
                            dps[:, i * PADW: i * PADW + NA],
                            stage[:, i * P: (i + 1) * P],
                            waug_sb[:],
                            start=True, stop=True,
                        )
                    tstage = dsp.tile([P, gb, RW], BF16, tag="tstage")
                    nc.gpsimd.memset(tstage[:], 0)
                    dv = dps[:].rearrange("p (i w) -> p i w", w=PADW)
                    nc.vector.tensor_copy(tstage[:, :ng, 0:OUTW],
                                          dv[:, :ng, 0:OUTW])
                    if not mean_heads:
                        tf32 = tstage[:].bitcast(F32)
                        nc.vector.tensor_copy(tf32[:, :ng, 32:36],
                                              dv[:, :ng, 64:68])
                    for (a, b, r) in table_row_ranges(g0 * P, g1 * P):
                        # emit aligned middle as one DMA; partial chunks solo
                        n0 = a
                        while n0 < b:
                            if n0 % P != 0 or b - n0 < P:
                                n1 = min(b, n0 - n0 % P + P)
                                ci = n0 // P - g0
                                nc.sync.dma_start(
                                    table[r + n0 - a: r + n1 - a, :],
                                    tstage[n0 % P: n0 % P + (n1 - n0), ci, :],
                                )
                            else:
                                n1 = n0 + (b - n0) // P * P
                                ci = n0 // P - g0
                                m = (n1 - n0) // P
                                nc.sync.dma_start(
                                    table[r + n0 - a: r + n1 - a, :].rearrange(
                                        "(i p) w -> p i w", p=P),
                                    tstage[:, ci: ci + m, :],
                                )
                            n0 = n1
                sent_sb = dsp.tile([2, RW], BF16, tag="sent")
                nc.sync.dma_start(sent_sb[:], sent_in[:])
                nc.sync.dma_start(table[HALF: HALF + 1, :], sent_sb[0:1, :])
                nc.sync.dma_start(table[N + 1: N + 2, :], sent_sb[1:2, :])

                ndc = Dp // P
                alps = pal.tile([P, ndc * 4], F32, space="PSUM")
                for i in range(ndc):
                    nc.tensor.matmul(
                        alps[:, i * 4: (i + 1) * 4],
                        hTow_sb[:, i * P: (i + 1) * P],
                        waug_sb[:, NA - 4: NA],
                        start=True, stop=True,
                    )
                alsb = dsp.tile([P, ndc * 4], F32, tag="alsb")
                nc.vector.tensor_copy(alsb[:], alps[:])
                nc.sync.dma_start(
                    aldv_d[:].rearrange("(i p) h -> p i h", p=P),
                    alsb[:].rearrange("p (i h) -> p i h", h=4),
                )

            # ---------------- edge phase
            with (
                tc.tile_pool(name="gpool", bufs=2) as gp,
                tc.tile_pool(name="spool", bufs=2) as ssp,
                tc.tile_pool(name="pwin", bufs=3, space="PSUM") as pw,
                tc.tile_pool(name="pex", bufs=3 if mean_heads else 1,
                             space="PSUM") as px,
                tc.tile_pool(name="palde", bufs=1 if mean_heads else 2,
                             space="PSUM") as pa,
                tc.tile_pool(name="psk", bufs=1 if mean_heads else 2,
                             space="PSUM") as pk,
            ):
                win_ps = {}
                ex_ps = {}
                for (s0, s1) in groups:
                    T = s1 - s0
                    g_lo = gp.tile([P, tcap, RW], BF16, tag="Glo")
                    g_hi = gp.tile([P, tcap, RW], BF16, tag="Ghi")
                    il_t = ssp.tile([P, tcap * 8], I16, tag="il")
                    ih_t = ssp.tile([P, tcap * 8], I16, tag="ih")
                    nc.sync.dma_start(il_t[:], idx_lo[:, s0 * 8:(s0 + tcap) * 8])
                    nc.sync.dma_start(ih_t[:], idx_hi[:, s0 * 8:(s0 + tcap) * 8])
                    if dbg_stage < 2:
                        continue
                    nc.gpsimd.dma_gather(
                        g_lo[:], table[0: HALF + 1, :],
                        il_t[:], tcap * P, tcap * P, RW,
                        single_packet=False)
                    nc.gpsimd.dma_gather(
                        g_hi[:], table[HALF + 1: TROWS, :],
                        ih_t[:], tcap * P, tcap * P, RW,
                        single_packet=False)

                    if dbg_stage < 3:
                        continue
                    alde = ssp.tile([P, tcap, 4], F32, tag="alde")
                    i = s0
                    while i < s1:
                        k = slabs[i][0]
                        j = i
                        while j < s1 and slabs[j][0] == k:
                            j += 1
                        q = P // k
                        run = j - i
                        vb = slabs[i][1]
                        cont = ssp.tile([16, tcap, 4], F32, tag="cont")
                        nc.sync.dma_start(
                            cont[:q, :run, :],
                            aldv_d[vb: vb + run * q, :].rearrange(
                                "(t j) h -> j t h", j=q),
                        )
                        aps = pa.tile([P, tcap * 4], F32, space="PSUM",
                                      tag="aldeps")
                        nc.tensor.matmul(
                            aps[:, : run * 4],
                            rep_sb[:q, KLIST.index(k), :],
                            cont[:q, :run, :].rearrange("j t h -> j (t h)"),
                            start=True, stop=True,
                        )
                        nc.vector.tensor_copy(
                            alde[:, i - s0: j - s0, :],
                            aps[:, : run * 4].rearrange("p (t h) -> p t h", h=4),
                        )
                        i = j

                    if dbg_stage < 4:
                        continue
                    if mean_heads:
                        ex_t = ssp.tile([P, 2 * tcap, 4], BF16, tag="ex")
                        tmp_t = ssp.tile([P, tcap // 2, 256], BF16, tag="tmp")
                    z_t = ssp.tile([P, 2 * tcap, 4], F32, tag="z")
                    for h in range(2):
                        gs = (g_lo if h == 0 else g_hi)[:, :T, :]
                        zs = z_t[:, h * tcap: h * tcap + T, :]
                        if mean_heads:
                            hc = tcap // 2
                            for c0 in range(0, T, hc):
                                c1 = min(T, c0 + hc)
                                nc.vector.tensor_tensor(
                                    tmp_t[:, : c1 - c0, :], gs[:, c0:c1, :],
                                    a2s_sb[:].to_broadcast([P, c1 - c0, 256]),
                                    AluOpType.mult)
                                nc.vector.reduce_sum(
                                    zs[:, c0:c1, :],
                                    tmp_t[:, : c1 - c0, :].rearrange(
                                        "p t (g c) -> p t g c", c=64),
                                    axis=mybir.AxisListType.X)
                        else:
                            gf = (g_lo if h == 0 else g_hi)[:].bitcast(F32)
                            nc.vector.tensor_copy(zs, gf[:, :T, 32:36])
                        nc.vector.tensor_tensor(zs, zs, alde[:, :T, :],
                                                AluOpType.add)
                        nc.vector.scalar_tensor_tensor(
                            zs, zs, 0.2, zs, AluOpType.mult, AluOpType.max)
                        if mean_heads:
                            nc.scalar.activation(
                                ex_t[:, h * tcap: h * tcap + T, :], zs,
                                mybir.ActivationFunctionType.Exp)
                        else:
                            nc.scalar.activation(
                                gs[:, :, 64:68], zs,
                                mybir.ActivationFunctionType.Exp)
                        for hh in range(4):
                            ex_ap = (ex_t[:, h * tcap: h * tcap + T, hh:hh + 1]
                                     if mean_heads
                                     else gs[:, :, 64 + hh: 65 + hh])
                            nc.vector.tensor_tensor(
                                gs[:, :, hh * (OUTW // 4):
                                   (hh + 1) * (OUTW // 4)],
                                gs[:, :, hh * (OUTW // 4):
                                   (hh + 1) * (OUTW // 4)],
                                ex_ap.to_broadcast([P, T, OUTW // 4]),
                                AluOpType.mult,
                            )

                    if dbg_stage < 5:
                        continue
                    for i in range(s0, s1):
                        k, vd0 = slabs[i]
                        w = slab_win[i]
                        if w not in win_ps:
                            win_ps[w] = pw.tile([P, NAW], F32, space="PSUM",
                                                tag="win", name=f"win{w}")
                            if mean_heads:
                                ex_ps[w] = px.tile([P, 4], F32, space="PSUM",
                                                   tag="exw", name=f"exw{w}")
                        q = P // k
                        sv = svar_sb[:, SVAR_INDEX[(k, (vd0 % P) // q)], :]
                        for h in range(2):
                            st = (h == 0) and (first_slab[w] == i)
                            fin = (h == 1) and (last_slab[w] == i)
                            gh = g_lo if h == 0 else g_hi
                            t = i - s0
                            if mean_heads:
                                nc.tensor.matmul(
                                    win_ps[w][:, 0:OUTW], sv, gh[:, t, :],
                                    start=st, stop=fin, skip_group_check=True)
                                nc.tensor.matmul(
                                    ex_ps[w][:], sv,
                                    ex_t[:, h * tcap + t, :],
                                    start=st, stop=fin, skip_group_check=True)
                            else:
                                nc.tensor.matmul(
                                    win_ps[w][:, 0:NAW], sv, gh[:, t, 0:NAW],
                                    start=st, stop=fin, skip_group_check=True)
                        if last_slab[w] != i or dbg_stage < 6:
                            continue
                        pwin = win_ps.pop(w)
                        pex = ex_ps.pop(w) if mean_heads else None
                        sk = pk.tile([P, 64], F32, space="PSUM", tag="skps")
                        nc.tensor.matmul(
                            sk[:], hTow_sb[:, w * P: (w + 1) * P], skipw_sb[:],
                            start=True, stop=True)
                        rec = ssp.tile([P, 4], F32, tag="rec")
                        nc.vector.reciprocal(
                            rec[:], pex[:] if mean_heads else pwin[:, OUTW:NAW])
                        yw = y_sb[:, w, :]
                        if mean_heads:
                            m_t = ssp.tile([P, 4, 64], F32, tag="mt")
                            for hh in range(4):
                                nc.vector.tensor_tensor(
                                    m_t[:, hh, :],
                                    pwin[:, hh * 64: (hh + 1) * 64],
                                    rec[:, hh: hh + 1].to_broadcast([P, 64]),
                                    AluOpType.mult)
                            nc.vector.tensor_tensor(yw, m_t[:, 0, :],
                                                    m_t[:, 1, :], AluOpType.add)
                            nc.vector.tensor_tensor(yw, yw, m_t[:, 2, :],
                                                    AluOpType.add)
                            nc.vector.tensor_tensor(yw, yw, m_t[:, 3, :],
                                                    AluOpType.add)
                            nc.vector.tensor_scalar_mul(yw, yw, 0.25)
                            nc.vector.tensor_tensor(yw, yw, sk[:], AluOpType.add)
                            nc.vector.tensor_tensor(yw, yw, bias_sb[:],
                                                    AluOpType.add)
                        else:
                            for hh in range(4):
                                nc.vector.tensor_tensor(
                                    yw[:, hh * 16: (hh + 1) * 16],
                                    pwin[:, hh * 16: (hh + 1) * 16],
                                    rec[:, hh: hh + 1].to_broadcast([P, 16]),
                                    AluOpType.mult)
                            nc.vector.tensor_tensor(yw, yw, sk[:], AluOpType.add)
                            nc.vector.tensor_tensor(yw, yw, bias_sb[:],
                                                    AluOpType.add)
                            if relu:
                                nc.vector.tensor_scalar_max(yw, yw, 0.0)

            nc.sync.dma_start(
                y_out[:].rearrange("(w p) c -> p w c", p=P), y_sb[:])
    nc.compile()
    return nc


# ------------------------------------------------------------------ driver

_CACHE = {}
_DBG = []
_EXEC_NS = []
_RESULTS = []


def _blockdiag(a):
    H, C = a.shape
    m = np.zeros((H * C, H), np.float32)
    for hh in range(H):
        m[hh * C: (hh + 1) * C, hh] = a[hh]
    return m


def _sent01():
    row = np.zeros((2, 64), np.float32)
    row[:, 32:36] = SENT_ALS
    return row.view(np.uint16).view(ml_dtypes.bfloat16)  # [2, 128]


def kernel(**inp):
    x = np.asarray(inp["x"], np.float32)
    ei = np.asarray(inp["edge_index"], np.int64)
    N, IN = x.shape
    E = ei.shape[1]

    loops = np.arange(N, dtype=np.int64)
    src = np.concatenate([ei[0], loops])
    dst = np.concatenate([ei[1], loops])

    pkey = ("plan", N, E, hash(ei.tobytes()))
    if pkey not in _CACHE:
        _CACHE[pkey] = build_plan(src, dst, N)
    shared, plans = _CACHE[pkey]
    Dp, D = shared["Dp"], shared["D"]
    Npad = _round_up(N, P)

    def prep01(Wv, a_s, a_d, cb, sW, sb, g, b, m, v):
        Wv, sW = np.asarray(Wv, np.float32), np.asarray(sW, np.float32)
        bns = (np.asarray(g) / np.sqrt(np.asarray(v) + EPS)).astype(np.float32)
        bnt = (np.asarray(b) - np.asarray(m) * bns).astype(np.float32)
        Waug = np.concatenate(
            [Wv * bns[None, :], Wv @ _blockdiag(np.asarray(a_s)),
             Wv @ _blockdiag(np.asarray(a_d))], 1)
        return (Waug, sW * bns[None, :], np.asarray(cb) * bns
                + np.asarray(sb) * bns + bnt, _sent01(), None)

    def prep2(Wv, a_s, a_d, cb, sW, sb):
        Wv = np.asarray(Wv, np.float32)
        a_s = np.asarray(a_s, np.float32)
        Waug = np.concatenate([Wv, Wv @ _blockdiag(np.asarray(a_d))], 1)
        hsent = np.zeros(256, np.float32)
        for hh in range(4):
            a = a_s[hh]
            hsent[hh * 64: (hh + 1) * 64] = SENT_ALS * a / (a * a).sum()
        sent = np.tile(hsent.astype(ml_dtypes.bfloat16), (2, 1))
        a2s_rep = np.tile(a_s.reshape(1, 1, 256).astype(ml_dtypes.bfloat16),
                          (P, 1, 1))
        return (Waug, np.asarray(sW, np.float32),
                np.asarray(cb) + np.asarray(sb), sent, a2s_rep)

    Ls = [
        prep01(inp["conv0_W"], inp["conv0_as"], inp["conv0_ad"], inp["conv0_b"],
               inp["skip0_W"], inp["skip0_b"], inp["bn0_g"], inp["bn0_b"],
               inp["bn0_m"], inp["bn0_v"]),
        prep01(inp["conv1_W"], inp["conv1_as"], inp["conv1_ad"], inp["conv1_b"],
               inp["skip1_W"], inp["skip1_b"], inp["bn1_g"], inp["bn1_b"],
               inp["bn1_m"], inp["bn1_v"]),
        prep2(inp["conv2_W"], inp["conv2_as"], inp["conv2_ad"], inp["conv2_b"],
              inp["skip2_W"], inp["skip2_b"]),
    ]

    rep = np.zeros((len(KLIST), 16, P), np.float32)
    for ki, k in enumerate(KLIST):
        for p in range(P):
            rep[ki, p // k, p] = 1.0
    rep_np = rep.reshape(len(KLIST) * 16, P)
    svar_np = SVAR_NP.astype(ml_dtypes.bfloat16)

    h = x
    for li in range(3):
        F = IN if li == 0 else 64
        OUTW = 64 if li < 2 else 256
        mean_heads = li == 2
        Waug, skipWf, biasv, sent, a2s_rep = Ls[li]
        lkey = ("nc", li, F, OUTW, N, E)
        if lkey not in _CACHE:
            _CACHE[lkey] = build_layer(
                shared, F, OUTW, relu=not mean_heads, mean_heads=mean_heads,
                tcap=64 if not mean_heads else 32,
                gb=8 if not mean_heads else 3)
        nck = _CACHE[lkey]

        hT_full = np.zeros((F, Npad), np.float32)
        hT_full[:, :N] = h.T
        base = {
            "hT": hT_full,
            "Waug": Waug.astype(np.float32),
            "skipW": skipWf.astype(np.float32),
            "biasR": np.tile(biasv.astype(np.float32), (P, 1)),
            "svar": svar_np,
            "rep": rep_np,
            "sent": np.asarray(sent, ml_dtypes.bfloat16),
        }
        if mean_heads:
            base["a2s"] = a2s_rep
        in_maps = []
        for c in range(NC):
            vmap = plans[c]["vmap"]
            hTow = np.zeros((F, Dp), np.float32)
            valid = vmap >= 0
            hTow[:, valid] = h[c * D + vmap[valid]].T
            in_maps.append(dict(base, hTow=hTow,
                                idx_lo=plans[c]["idx_lo"],
                                idx_hi=plans[c]["idx_hi"]))
        import time as _time
        _t0 = _time.time()
        res = run_bass_kernel_spmd(nck, in_maps, core_ids=list(range(NC)))
        _RESULTS.append(res)
        if res.exec_time_ns:
            _EXEC_NS.append(res.exec_time_ns)
        print(f"  layer {li} run wall: {_time.time()-_t0:.1f}s", flush=True)
        hn = np.zeros((N, 64), np.float32)
        for c in range(NC):
            vmap = plans[c]["vmap"]
            valid = vmap >= 0
            hn[c * D + vmap[valid]] = res.results[c]["y"][valid]
        h = hn
        _DBG.append(h)
    return h



# revision 11
# speedup vs baseline: 2.7483x; 2.7483x over previous
"""GAT (3-layer, PyG-style) Trainium2 Bass kernel, 8-core dst-sharded. v2.

Self-contained: takes full inputs, shards internally, returns full output.

Design (per layer -> two SPMD launches):
  Launch A (dense, src-sharded): each core computes its 1/8 of the gather
    table rows [64 feat bf16 | 4 al_src f32] = 256B/row via PE matmuls with
    Waug = [W(*bns) | W@blk(a_src) | W@blk(a_dst)], plus per-dst al_dst and
    the dense skip+bias rows for its dst shard. Host assembles the full
    table (plus 2 sentinel rows) and expands the per-slot al_dst stream.
  Launch B (edge, dst-sharded): exact-CSR edge slots (chunks of 128 slots,
    one (window, src-half) per chunk; schedule shared across cores, per-core
    counts balanced by an LPT assignment of dst nodes to (core, window)).
    Per group of chunks: dma_gather of 256B rows (4 SWDGE queues round-
    robin), ex = exp(leaky_relu(al_s + al_d)) on DVE/Act, messages scaled
    by ex, segment-sum via per-chunk one-hot matmuls (sel built on DVE by
    comparing a shipped dstloc column against an iota row) accumulating
    [msg | sum_ex] per 128-dst window in PSUM, drained into an SBUF
    accumulator. One batched normalize pass at the end.
  Layer 2 uses GAT linearity: segment-sum runs in h1-space (same 256B rows)
    with 4 per-head ex-scaled copies; W2 is applied per window afterwards
    (PE transpose + 2 matmuls), then mean over heads.
"""
import numpy as np
import ml_dtypes

import concourse.bacc as bacc
import concourse.mybir as mybir
import concourse.tile as tile
from concourse.alu_op_type import AluOpType
from concourse.bass_utils import run_bass_kernel_spmd

BF16 = mybir.dt.bfloat16
F32 = mybir.dt.float32
I16 = mybir.dt.int16

NC = 8
P = 128
EPS = 1e-5
SENT_ALS = -40.0
NQ = 4          # SWDGE queues
GC01 = 64       # chunks per gather group, layers 0/1
GC2 = 24        # chunks per gather group, layer 2

N_NODES = 50000
SB = N_NODES // NC          # src nodes per core (6250)
NWIN = (SB + P - 1) // P    # dst windows per core (49)
DP = NWIN * P               # padded dst slots per core (6272)
RB = DP                     # table row block per core (6272)
LO_ROWS = 4 * RB            # rows in the lo half (25088)
SENT_REL = LO_ROWS          # sentinel index relative to each half base
TROWS = 2 * (LO_ROWS + 1)


def _ceil(a, b):
    return -(-a // b)


# ----------------------------------------------------------------- planning

def build_plan(src, dst, N):
    assert N == N_NODES
    # src -> permuted table row (relative to half base) and half flag
    s_core = src // SB
    arow = (src % SB) + s_core * RB
    half = (s_core >= 4).astype(np.int64)
    rel = np.where(half == 0, arow, arow - 4 * RB)

    # balanced assignment of dst nodes to (core, window) buckets
    deg = np.zeros((N, 2), np.int64)
    np.add.at(deg, (dst, half), 1)
    tot = deg.sum(1)
    order = np.argsort(-tot, kind="stable")
    import heapq
    heap = [(0, c * NWIN + w) for c in range(NC) for w in range(NWIN)]
    heapq.heapify(heap)
    bcount = np.zeros(NC * NWIN, np.int64)
    dst_c = np.empty(N, np.int64)
    dst_w = np.empty(N, np.int64)
    dst_p = np.empty(N, np.int64)
    pending = []
    for n in order:
        while True:
            load, b = heapq.heappop(heap)
            if bcount[b] < P:
                break
        dst_c[n] = b // NWIN
        dst_w[n] = b % NWIN
        dst_p[n] = bcount[b]
        bcount[b] += 1
        if bcount[b] < P:
            heapq.heappush(heap, (load + tot[n], b))
        else:
            pending.append(b)

    vmap = np.full((NC, DP), -1, np.int64)
    vmap[dst_c, dst_w * P + dst_p] = np.arange(N)

    # per (core, window, half) edge counts -> shared chunk schedule
    ec = dst_c[dst]
    ew = dst_w[dst]
    epos = dst_p[dst]
    cnt = np.zeros((NC, NWIN, 2), np.int64)
    np.add.at(cnt, (ec, ew, half), 1)
    nch = np.zeros((NWIN, 2), np.int64)
    for w in range(NWIN):
        for h in range(2):
            nch[w, h] = _ceil(int(cnt[:, w, h].max()), P) if cnt[:, w, h].max() else 0
    chunks = []          # (w, h) in schedule order: lo sweep then hi sweep
    for h in range(2):
        for w in range(NWIN):
            chunks += [(w, h)] * int(nch[w, h])
    nchunks = len(chunks)
    slot_base = {}       # (w, h) -> first slot index in the combined stream
    o = 0
    for (w, h) in chunks:
        slot_base.setdefault((w, h), o * P)
        o += 1
    nlo = int(nch[:, 0].sum())
    SLO, SHI = nlo * P, (nchunks - nlo) * P

    # per-core streams
    plans = []
    for c in range(NC):
        em = ec == c
        eh = half[em]
        key = eh * (NWIN * P) + ew[em] * P  # per (h, w) group key base
        # stable ordering by (h, w); position within group via argsort
        si = np.argsort(key, kind="stable")
        erel = rel[em][si]
        ewk = ew[em][si]
        ehk = eh[si]
        epk = epos[em][si]
        idx_all = np.full(nchunks * P, SENT_REL, np.int64)
        dloc = np.full(nchunks * P, -1.0, np.float32)
        sdst = np.zeros(nchunks * P, np.int64)
        # group runs: edges sorted by (h, w); fill each group's slot range
        bounds = np.searchsorted(
            ehk * NWIN + ewk, np.arange(2 * NWIN + 1))
        for h in range(2):
            for w in range(NWIN):
                a, b = bounds[h * NWIN + w], bounds[h * NWIN + w + 1]
                if a == b:
                    continue
                s0 = slot_base[(w, h)]
                m = b - a
                idx_all[s0: s0 + m] = erel[a:b]
                dloc[s0: s0 + m] = epk[a:b]
                sdst[s0: s0 + m] = w * P + epk[a:b]
        idx_lo = _wrap16(idx_all[:SLO])
        idx_hi = _wrap16(idx_all[SLO:])
        dstloc = dloc.reshape(nchunks, P).T.astype(ml_dtypes.bfloat16)
        sdst_w = sdst.reshape(nchunks, P).transpose(1, 0)
        plans.append(dict(vmap=vmap[c], idx_lo=idx_lo, idx_hi=idx_hi,
                          dstloc=np.ascontiguousarray(dstloc),
                          slot_dst=np.ascontiguousarray(sdst_w)))
    shared = dict(chunks=chunks, nchunks=nchunks, nlo=nlo, SLO=SLO, SHI=SHI)
    return shared, plans


def _wrap16(stream):
    S = len(stream)
    w = stream.reshape(S // 16, 16).T.astype(np.int16)
    return np.ascontiguousarray(np.tile(w, (8, 1)))


PERMC = (np.arange(DP) % P) * NWIN + np.arange(DP) // P  # col k -> slot p*49+i


# ------------------------------------------------------------- launch A (dense)

def build_dense(F):
    nc = bacc.Bacc("TRN2", target_bir_lowering=False, debug=False)
    hTs = nc.dram_tensor("hTs", [F, DP], BF16, kind="ExternalInput")
    hTow = nc.dram_tensor("hTow", [F, DP], BF16, kind="ExternalInput")
    Waug = nc.dram_tensor("Waug", [F, 72], BF16, kind="ExternalInput")
    skipW = nc.dram_tensor("skipW", [F, 64], BF16, kind="ExternalInput")
    biasR = nc.dram_tensor("biasR", [P, 64], F32, kind="ExternalInput")
    tshard = nc.dram_tensor("tshard", [DP, P], BF16, kind="ExternalOutput")
    aldv = nc.dram_tensor("aldv", [DP, 4], F32, kind="ExternalOutput")
    skipd = nc.dram_tensor("skipd", [DP, 64], F32, kind="ExternalOutput")

    with tile.TileContext(nc) as tc:
        with (
            tc.tile_pool(name="c", bufs=1) as cp,
            tc.tile_pool(name="ps", bufs=2, space="PSUM") as pp,
        ):
            hts_sb = cp.tile([F, DP], BF16)
            nc.sync.dma_start(hts_sb[:], hTs[:])
            htow_sb = cp.tile([F, DP], BF16)
            nc.sync.dma_start(htow_sb[:], hTow[:])
            waug_sb = cp.tile([F, 72], BF16)
            nc.sync.dma_start(waug_sb[:], Waug[:])
            skipw_sb = cp.tile([F, 64], BF16)
            nc.sync.dma_start(skipw_sb[:], skipW[:])
            bias_sb = cp.tile([P, 64], F32)
            nc.sync.dma_start(bias_sb[:], biasR[:])

            tstage = cp.tile([P, NWIN, P], BF16)
            asb = cp.tile([P, NWIN, 4], F32)
            ssb = cp.tile([P, NWIN, 64], F32)
            nc.gpsimd.memset(tstage[:], 0)
            for i in range(NWIN):
                dps = pp.tile([P, 72], F32, space="PSUM", tag="dps")
                nc.tensor.matmul(dps[:], hts_sb[:, i * P:(i + 1) * P],
                                 waug_sb[:], start=True, stop=True)
                nc.vector.tensor_copy(tstage[:, i, 0:64], dps[:, 0:64])
                tf32 = tstage[:].bitcast(F32)
                nc.vector.tensor_copy(tf32[:, i, 32:36], dps[:, 64:68])
                aps = pp.tile([P, 4], F32, space="PSUM", tag="aps")
                nc.tensor.matmul(aps[:], htow_sb[:, i * P:(i + 1) * P],
                                 waug_sb[:, 68:72], start=True, stop=True)
                sps = pp.tile([P, 64], F32, space="PSUM", tag="sps")
                nc.tensor.matmul(sps[:], htow_sb[:, i * P:(i + 1) * P],
                                 skipw_sb[:], start=True, stop=True)
                nc.vector.tensor_copy(asb[:, i, :], aps[:])
                nc.vector.tensor_tensor(ssb[:, i, :], sps[:],
                                        bias_sb[:], AluOpType.add)
            nc.sync.dma_start(
                tshard[:].rearrange("(p i) w -> p i w", p=P), tstage[:])
            nc.sync.dma_start(
                aldv[:].rearrange("(p i) h -> p i h", p=P), asb[:])
            nc.sync.dma_start(
                skipd[:].rearrange("(p i) c -> p i c", p=P), ssb[:])
    nc.compile()
    return nc


# ------------------------------------------------------------- launch B (edge)

def build_edge(shared, l2):
    chunks = shared["chunks"]
    nchunks = shared["nchunks"]
    nlo = shared["nlo"]
    SLO, SHI = shared["SLO"], shared["SHI"]
    GC = GC2 if l2 else GC01
    NW = 260 if l2 else 68

    nc = bacc.Bacc("TRN2", target_bir_lowering=False, debug=False,
                   num_swdge_queues=NQ)
    table = nc.dram_tensor("table", [TROWS, P], BF16, kind="ExternalInput")
    idx_lo = nc.dram_tensor("idx_lo", [P, max(SLO, 16) // 16], I16,
                            kind="ExternalInput")
    idx_hi = nc.dram_tensor("idx_hi", [P, max(SHI, 16) // 16], I16,
                            kind="ExternalInput")
    dstloc = nc.dram_tensor("dstloc", [P, nchunks], BF16,
                            kind="ExternalInput")
    alde_in = nc.dram_tensor("alde", [P, nchunks, 4], F32,
                             kind="ExternalInput")
    skipd_in = nc.dram_tensor("skipd", [DP, 64], F32, kind="ExternalInput")
    iota_in = nc.dram_tensor("iota", [P, P], BF16, kind="ExternalInput")
    if l2:
        w2_in = nc.dram_tensor("w2", [P, 2, 64], BF16, kind="ExternalInput")
        ident_in = nc.dram_tensor("ident", [P, P], BF16, kind="ExternalInput")
    y_out = nc.dram_tensor("y", [DP, 64], F32, kind="ExternalOutput")

    # group schedule: runs of <= GC chunks, same half
    groups = []
    k = 0
    while k < nchunks:
        k1 = min(k + GC, nchunks, nlo if k < nlo else nchunks)
        groups.append((k, k1))
        k = k1
    first = [False] * nchunks
    last = [False] * nchunks
    seen = set()
    for k, (w, h) in enumerate(chunks):
        if (h, w) not in seen:
            seen.add((h, w))
            first[k] = True
        if k + 1 >= nchunks or chunks[k + 1] != (w, h):
            last[k] = True

    with tile.TileContext(nc) as tc:
        with (
            tc.tile_pool(name="c", bufs=1) as cp,
            tc.tile_pool(name="g", bufs=3) as gp,
            tc.tile_pool(name="s", bufs=3) as sp,
            tc.tile_pool(name="pw", bufs=4, space="PSUM") as pw,
            tc.tile_pool(name="pt", bufs=2, space="PSUM") as pt,
        ):
            dloc_sb = cp.tile([P, nchunks], BF16)
            nc.sync.dma_start(dloc_sb[:], dstloc[:])
            alde_sb = cp.tile([P, nchunks, 4], F32)
            nc.sync.dma_start(alde_sb[:], alde_in[:])
            skipd_sb = cp.tile([P, NWIN, 64], F32)
            nc.sync.dma_start(
                skipd_sb[:], skipd_in[:].rearrange("(i p) c -> p i c", p=P))
            iota_sb = cp.tile([P, 1, P], BF16)
            nc.sync.dma_start(iota_sb[:, 0, :], iota_in[:])
            if l2:
                w2_sb = cp.tile([P, 2, 64], BF16)
                nc.sync.dma_start(w2_sb[:], w2_in[:])
                ident_sb = cp.tile([P, P], BF16)
                nc.sync.dma_start(ident_sb[:], ident_in[:])
            msum = cp.tile([P, NWIN, NW], F32)
            nc.gpsimd.memset(msum[:], 0)
            y_sb = cp.tile([P, NWIN, 64], F32)

            win_ps = {}
            for gi, (k0, k1) in enumerate(groups):
                T = k1 - k0
                h = chunks[k0][1]
                base = table[0: LO_ROWS + 1, :] if h == 0 \
                    else table[LO_ROWS + 1: TROWS, :]
                o16 = (k0 * P if h == 0 else (k0 - nlo) * P) // 16
                idx_t = sp.tile([P, GC * 8], I16, tag="idx")
                nc.sync.dma_start(
                    idx_t[:, : T * 8],
                    (idx_lo if h == 0 else idx_hi)[:, o16: o16 + T * 8])
                gt = gp.tile([P, GC, P], BF16, tag="g")
                nc.gpsimd.dma_gather(
                    gt[:, :T, :], base, idx_t[:, : T * 8], T * P, T * P, P,
                    single_packet=False, queue_num=gi % NQ)

                zf = sp.tile([P, GC, 4], F32, tag="z")
                gf = gt[:].bitcast(F32)
                nc.vector.tensor_copy(zf[:, :T, :], gf[:, :T, 32:36])
                nc.vector.tensor_tensor(zf[:, :T, :], zf[:, :T, :],
                                        alde_sb[:, k0:k1, :], AluOpType.add)
                nc.vector.scalar_tensor_tensor(
                    zf[:, :T, :], zf[:, :T, :], 0.2, zf[:, :T, :],
                    AluOpType.mult, AluOpType.max)
                exb = sp.tile([P, GC, 4], BF16, tag="ex")
                nc.scalar.activation(exb[:, :T, :], zf[:, :T, :],
                                     mybir.ActivationFunctionType.Exp)
                sel = sp.tile([P, GC, P], BF16, tag="sel")
                nc.vector.tensor_tensor(
                    sel[:, :T, :],
                    iota_sb[:].to_broadcast([P, T, P]),
                    dloc_sb[:, k0:k1, None].to_broadcast([P, T, P]),
                    AluOpType.is_equal)

                if l2:
                    rhs = gp.tile([P, GC, 260], BF16, tag="rhs")
                    nc.vector.tensor_tensor(
                        rhs[:, :T, 0:256].rearrange(
                            "p t (h c) -> p t h c", c=64),
                        gt[:, :T, None, 0:64].to_broadcast([P, T, 4, 64]),
                        exb[:, :T, :, None].to_broadcast([P, T, 4, 64]),
                        AluOpType.mult)
                    nc.vector.tensor_copy(rhs[:, :T, 256:260], exb[:, :T, :])
                else:
                    nc.vector.tensor_tensor(
                        gt[:, :T, 0:64].rearrange("p t (h c) -> p t h c", c=16),
                        gt[:, :T, 0:64].rearrange("p t (h c) -> p t h c", c=16),
                        exb[:, :T, :, None].to_broadcast([P, T, 4, 16]),
                        AluOpType.mult)
                    nc.vector.tensor_copy(gt[:, :T, 64:68], exb[:, :T, :])

                for t in range(T):
                    k = k0 + t
                    w, hh = chunks[k]
                    if first[k]:
                        win_ps[w] = pw.tile([P, NW], F32, space="PSUM",
                                            tag="win", name=f"win{w}h{hh}")
                    rhs_ap = rhs[:, t, :] if l2 else gt[:, t, 0:68]
                    nc.tensor.matmul(win_ps[w][:], sel[:, t, :], rhs_ap,
                                     start=first[k], stop=last[k],
                                     skip_group_check=True)
                    if last[k]:
                        pwin = win_ps.pop(w)
                        nc.vector.tensor_tensor(msum[:, w, :], msum[:, w, :],
                                                pwin[:], AluOpType.add)

            # ---------------- batched normalize + output
            rec = cp.tile([P, NWIN, 4], F32)
            nc.vector.reciprocal(rec[:], msum[:, :, NW - 4: NW])
            if l2:
                snorm = cp.tile([P, NWIN, 256], BF16)
                nc.vector.tensor_tensor(
                    snorm[:].rearrange("p w (h c) -> p w h c", c=64),
                    msum[:, :, 0:256].rearrange("p w (h c) -> p w h c", c=64),
                    rec[:, :, :, None].to_broadcast([P, NWIN, 4, 64]),
                    AluOpType.mult)
                for w in range(NWIN):
                    yps = pt.tile([P, 64], F32, space="PSUM", tag="yps")
                    for j in range(2):
                        tp = pt.tile([P, P], BF16, space="PSUM", tag="tp")
                        nc.tensor.matmul(
                            tp[:], snorm[:, w, j * P:(j + 1) * P],
                            ident_sb[:], is_transpose=True,
                            start=True, stop=True, skip_group_check=True)
                        st = sp.tile([P, P], BF16, tag="st")
                        nc.vector.tensor_copy(st[:], tp[:])
                        nc.tensor.matmul(yps[:], st[:], w2_sb[:, j, :],
                                         start=(j == 0), stop=(j == 1),
                                         skip_group_check=True)
                    nc.vector.scalar_tensor_tensor(
                        y_sb[:, w, :], yps[:], 0.25, skipd_sb[:, w, :],
                        AluOpType.mult, AluOpType.add)
            else:
                nc.vector.tensor_tensor(
                    y_sb[:].rearrange("p w (h c) -> p w h c", c=16),
                    msum[:, :, 0:64].rearrange("p w (h c) -> p w h c", c=16),
                    rec[:, :, :, None].to_broadcast([P, NWIN, 4, 16]),
                    AluOpType.mult)
                nc.vector.tensor_tensor(y_sb[:], y_sb[:], skipd_sb[:],
                                        AluOpType.add)
                nc.vector.tensor_scalar_max(y_sb[:], y_sb[:], 0.0)
            nc.sync.dma_start(
                y_out[:].rearrange("(i p) c -> p i c", p=P), y_sb[:])
    nc.compile()
    return nc


# ------------------------------------------------------------------ driver

_CACHE = {}
_DBG = []
_EXEC_NS = []
_RESULTS = []


def _blockdiag(a):
    H, C = a.shape
    m = np.zeros((H * C, H), np.float32)
    for hh in range(H):
        m[hh * C: (hh + 1) * C, hh] = a[hh]
    return m


def _bf(x):
    return np.ascontiguousarray(np.asarray(x, np.float32)
                                .astype(ml_dtypes.bfloat16))


def kernel(**inp):
    x = np.asarray(inp["x"], np.float32)
    ei = np.asarray(inp["edge_index"], np.int64)
    N, IN = x.shape
    E = ei.shape[1]

    loops = np.arange(N, dtype=np.int64)
    src = np.concatenate([ei[0], loops])
    dst = np.concatenate([ei[1], loops])

    pkey = ("plan", N, E, hash(ei.tobytes()))
    if pkey not in _CACHE:
        _CACHE[pkey] = build_plan(src, dst, N)
    shared, plans = _CACHE[pkey]

    def prep01(Wv, a_s, a_d, cb, sW, sb, g, b, m, v):
        Wv, sW = np.asarray(Wv, np.float32), np.asarray(sW, np.float32)
        bns = (np.asarray(g) / np.sqrt(np.asarray(v) + EPS)).astype(np.float32)
        bnt = (np.asarray(b) - np.asarray(m) * bns).astype(np.float32)
        Waug = np.concatenate(
            [Wv * bns[None, :], Wv @ _blockdiag(np.asarray(a_s)),
             Wv @ _blockdiag(np.asarray(a_d))], 1)
        return (Waug, sW * bns[None, :],
                (np.asarray(cb) + np.asarray(sb)) * bns + bnt, None)

    def prep2(Wv, a_s, a_d, cb, sW, sb):
        Wv = np.asarray(Wv, np.float32)
        Waug = np.concatenate(
            [np.eye(64, dtype=np.float32), Wv @ _blockdiag(np.asarray(a_s)),
             Wv @ _blockdiag(np.asarray(a_d))], 1)
        w2 = np.ascontiguousarray(
            Wv.reshape(64, 4, 64).transpose(1, 0, 2).reshape(256, 64)
            .reshape(2, 128, 64).transpose(1, 0, 2))
        return (Waug, np.asarray(sW, np.float32),
                np.asarray(cb) + np.asarray(sb), w2)

    Ls = [
        prep01(inp["conv0_W"], inp["conv0_as"], inp["conv0_ad"],
               inp["conv0_b"], inp["skip0_W"], inp["skip0_b"],
               inp["bn0_g"], inp["bn0_b"], inp["bn0_m"], inp["bn0_v"]),
        prep01(inp["conv1_W"], inp["conv1_as"], inp["conv1_ad"],
               inp["conv1_b"], inp["skip1_W"], inp["skip1_b"],
               inp["bn1_g"], inp["bn1_b"], inp["bn1_m"], inp["bn1_v"]),
        prep2(inp["conv2_W"], inp["conv2_as"], inp["conv2_ad"],
              inp["conv2_b"], inp["skip2_W"], inp["skip2_b"]),
    ]

    iota_np = np.tile(np.arange(P, dtype=np.float32), (P, 1)).astype(
        ml_dtypes.bfloat16)
    ident_np = np.eye(P, dtype=np.float32).astype(ml_dtypes.bfloat16)
    # sentinel row: zero feats, al_src = -40 (f32 packed in bf16 slots 64..71)
    sent_view = np.zeros(P, np.uint16)
    sent_view[64:72] = np.full(4, SENT_ALS, np.float32).view(np.uint16)
    sent = sent_view.view(ml_dtypes.bfloat16)

    h = x
    for li in range(3):
        F = IN if li == 0 else 64
        l2 = li == 2
        Waug, skipWf, biasv, w2 = Ls[li]
        akey = ("A", F)
        if akey not in _CACHE:
            _CACHE[akey] = build_dense(F)
        bkey = ("B", l2)
        if bkey not in _CACHE:
            _CACHE[bkey] = build_edge(shared, l2)

        hT = h.T.astype(np.float32)
        base_a = {
            "Waug": _bf(Waug),
            "skipW": _bf(skipWf),
            "biasR": np.tile(np.asarray(biasv, np.float32), (P, 1)),
        }
        a_maps = []
        for c in range(NC):
            node = c * SB + PERMC
            valid_s = PERMC < SB
            hts = np.zeros((F, DP), np.float32)
            hts[:, valid_s] = hT[:, node[valid_s]]
            vm = plans[c]["vmap"][PERMC]
            valid_d = vm >= 0
            htow = np.zeros((F, DP), np.float32)
            htow[:, valid_d] = hT[:, vm[valid_d]]
            a_maps.append(dict(base_a, hTs=_bf(hts), hTow=_bf(htow)))
        res_a = run_bass_kernel_spmd(_CACHE[akey], a_maps,
                                     core_ids=list(range(NC)))
        _RESULTS.append(res_a)
        if res_a.exec_time_ns:
            _EXEC_NS.append(res_a.exec_time_ns)

        tbl = np.empty((TROWS, P), ml_dtypes.bfloat16)
        for c in range(4):
            tbl[c * RB:(c + 1) * RB] = res_a.results[c]["tshard"]
        tbl[LO_ROWS] = sent
        for c in range(4, 8):
            tbl[LO_ROWS + 1 + (c - 4) * RB: LO_ROWS + 1 + (c - 3) * RB] = \
                res_a.results[c]["tshard"]
        tbl[TROWS - 1] = sent

        base_b = {"table": tbl, "iota": iota_np}
        if l2:
            base_b["w2"] = _bf(w2)
            base_b["ident"] = ident_np
        b_maps = []
        for c in range(NC):
            aldv = res_a.results[c]["aldv"]
            alde = aldv[plans[c]["slot_dst"]]  # [128, nchunks, 4]
            b_maps.append(dict(
                base_b,
                idx_lo=plans[c]["idx_lo"], idx_hi=plans[c]["idx_hi"],
                dstloc=plans[c]["dstloc"],
                alde=np.ascontiguousarray(alde.astype(np.float32)),
                skipd=np.ascontiguousarray(
                    res_a.results[c]["skipd"].astype(np.float32))))
        res_b = run_bass_kernel_spmd(_CACHE[bkey], b_maps,
                                     core_ids=list(range(NC)))
        _RESULTS.append(res_b)
        if res_b.exec_time_ns:
            _EXEC_NS.append(res_b.exec_time_ns)

        hn = np.zeros((N, 64), np.float32)
        for c in range(NC):
            vm = plans[c]["vmap"]
            valid = vm >= 0
            hn[vm[valid]] = res_b.results[c]["y"][valid]
        h = hn
        _DBG.append(h)
    return h


# revision 14
# speedup vs baseline: 3.6740x; 1.3368x over previous
"""GAT (3-layer, PyG-style) Trainium2 Bass kernel, 8-core dst-sharded. v2.

Self-contained: takes full inputs, shards internally, returns full output.

Design (per layer -> two SPMD launches):
  Launch A (dense, src-sharded): each core computes its 1/8 of the gather
    table rows [64 feat bf16 | 4 al_src f32] = 256B/row via PE matmuls with
    Waug = [W(*bns) | W@blk(a_src) | W@blk(a_dst)], plus per-dst al_dst and
    the dense skip+bias rows for its dst shard. Host assembles the full
    table (plus 2 sentinel rows) and expands the per-slot al_dst stream.
  Launch B (edge, dst-sharded): exact-CSR edge slots (chunks of 128 slots,
    one (window, src-half) per chunk; schedule shared across cores, per-core
    counts balanced by an LPT assignment of dst nodes to (core, window)).
    Per group of chunks: dma_gather of 256B rows (4 SWDGE queues round-
    robin), ex = exp(leaky_relu(al_s + al_d)) on DVE/Act, messages scaled
    by ex, segment-sum via per-chunk one-hot matmuls (sel built on DVE by
    comparing a shipped dstloc column against an iota row) accumulating
    [msg | sum_ex] per 128-dst window in PSUM, drained into an SBUF
    accumulator. One batched normalize pass at the end.
  Layer 2 uses GAT linearity: segment-sum runs in h1-space (same 256B rows)
    with 4 per-head ex-scaled copies; W2 is applied per window afterwards
    (PE transpose + 2 matmuls), then mean over heads.
"""
import numpy as np
import ml_dtypes

import concourse.bacc as bacc
import concourse.mybir as mybir
import concourse.tile as tile
from concourse.alu_op_type import AluOpType
from concourse.bass_utils import run_bass_kernel_spmd

BF16 = mybir.dt.bfloat16
F32 = mybir.dt.float32
I16 = mybir.dt.int16

NC = 8
P = 128
EPS = 1e-5
SENT_ALS = -40.0
NQ = 4          # SWDGE queues
GC01 = 64       # chunks per gather group, layers 0/1
GC2 = 24        # chunks per gather group, layer 2

N_NODES = 50000
SB = N_NODES // NC          # src nodes per core (6250)
NWIN = (SB + P - 1) // P    # dst windows per core (49)
DP = NWIN * P               # padded dst slots per core (6272)
RB = DP                     # table row block per core (6272)
LO_ROWS = 4 * RB            # rows in the lo half (25088)
SENT_REL = LO_ROWS          # sentinel index relative to each half base
TROWS = 2 * (LO_ROWS + 1)


def _ceil(a, b):
    return -(-a // b)


# ----------------------------------------------------------------- planning

def build_plan(src, dst, N):
    assert N == N_NODES
    # src -> permuted table row (relative to half base) and half flag
    s_core = src // SB
    arow = (src % SB) + s_core * RB
    half = (s_core >= 4).astype(np.int64)
    rel = np.where(half == 0, arow, arow - 4 * RB)

    # balanced assignment of dst nodes to (core, window) buckets
    deg = np.zeros((N, 2), np.int64)
    np.add.at(deg, (dst, half), 1)
    tot = deg.sum(1)
    order = np.argsort(-tot, kind="stable")
    import heapq
    heap = [(0, c * NWIN + w) for c in range(NC) for w in range(NWIN)]
    heapq.heapify(heap)
    bcount = np.zeros(NC * NWIN, np.int64)
    dst_c = np.empty(N, np.int64)
    dst_w = np.empty(N, np.int64)
    dst_p = np.empty(N, np.int64)
    pending = []
    for n in order:
        while True:
            load, b = heapq.heappop(heap)
            if bcount[b] < P:
                break
        dst_c[n] = b // NWIN
        dst_w[n] = b % NWIN
        dst_p[n] = bcount[b]
        bcount[b] += 1
        if bcount[b] < P:
            heapq.heappush(heap, (load + tot[n], b))
        else:
            pending.append(b)

    vmap = np.full((NC, DP), -1, np.int64)
    vmap[dst_c, dst_w * P + dst_p] = np.arange(N)

    # per (core, window, half) edge counts -> shared chunk schedule
    ec = dst_c[dst]
    ew = dst_w[dst]
    epos = dst_p[dst]
    cnt = np.zeros((NC, NWIN, 2), np.int64)
    np.add.at(cnt, (ec, ew, half), 1)
    nch = np.zeros((NWIN, 2), np.int64)
    for w in range(NWIN):
        for h in range(2):
            nch[w, h] = _ceil(int(cnt[:, w, h].max()), P) if cnt[:, w, h].max() else 0
    chunks = []          # (w, h) in schedule order: lo sweep then hi sweep
    for h in range(2):
        for w in range(NWIN):
            chunks += [(w, h)] * int(nch[w, h])
    nchunks = len(chunks)
    slot_base = {}       # (w, h) -> first slot index in the combined stream
    o = 0
    for (w, h) in chunks:
        slot_base.setdefault((w, h), o * P)
        o += 1
    nlo = int(nch[:, 0].sum())
    SLO, SHI = nlo * P, (nchunks - nlo) * P

    # per-core streams
    plans = []
    for c in range(NC):
        em = ec == c
        eh = half[em]
        key = eh * (NWIN * P) + ew[em] * P  # per (h, w) group key base
        # stable ordering by (h, w); position within group via argsort
        si = np.argsort(key, kind="stable")
        erel = rel[em][si]
        ewk = ew[em][si]
        ehk = eh[si]
        epk = epos[em][si]
        idx_all = np.full(nchunks * P, SENT_REL, np.int64)
        dloc = np.full(nchunks * P, -1.0, np.float32)
        sdst = np.zeros(nchunks * P, np.int64)
        # group runs: edges sorted by (h, w); fill each group's slot range
        bounds = np.searchsorted(
            ehk * NWIN + ewk, np.arange(2 * NWIN + 1))
        for h in range(2):
            for w in range(NWIN):
                a, b = bounds[h * NWIN + w], bounds[h * NWIN + w + 1]
                if a == b:
                    continue
                s0 = slot_base[(w, h)]
                m = b - a
                idx_all[s0: s0 + m] = erel[a:b]
                dloc[s0: s0 + m] = epk[a:b]
                sdst[s0: s0 + m] = w * P + epk[a:b]
        idx_lo = _wrap16(idx_all[:SLO])
        idx_hi = _wrap16(idx_all[SLO:])
        dstloc = dloc.reshape(nchunks, P).T.astype(ml_dtypes.bfloat16)
        sdst_w = sdst.reshape(nchunks, P).transpose(1, 0)
        plans.append(dict(vmap=vmap[c], idx_lo=idx_lo, idx_hi=idx_hi,
                          dstloc=np.ascontiguousarray(dstloc),
                          slot_dst=np.ascontiguousarray(sdst_w)))
    shared = dict(chunks=chunks, nchunks=nchunks, nlo=nlo, SLO=SLO, SHI=SHI)
    return shared, plans


def _wrap16(stream):
    S = len(stream)
    w = stream.reshape(S // 16, 16).T.astype(np.int16)
    return np.ascontiguousarray(np.tile(w, (8, 1)))


PERMC = (np.arange(DP) % P) * NWIN + np.arange(DP) // P  # col k -> slot p*49+i


# ------------------------------------------------------------- launch A (dense)

def build_dense(F):
    nc = bacc.Bacc("TRN2", target_bir_lowering=False, debug=False)
    hTs = nc.dram_tensor("hTs", [F, DP], BF16, kind="ExternalInput")
    hTow = nc.dram_tensor("hTow", [F, DP], BF16, kind="ExternalInput")
    Waug = nc.dram_tensor("Waug", [F, 72], BF16, kind="ExternalInput")
    skipW = nc.dram_tensor("skipW", [F, 64], BF16, kind="ExternalInput")
    biasR = nc.dram_tensor("biasR", [P, 64], F32, kind="ExternalInput")
    tshard = nc.dram_tensor("tshard", [DP, P], BF16, kind="ExternalOutput")
    aldv = nc.dram_tensor("aldv", [DP, 4], F32, kind="ExternalOutput")
    skipd = nc.dram_tensor("skipd", [DP, 64], F32, kind="ExternalOutput")

    with tile.TileContext(nc) as tc:
        with (
            tc.tile_pool(name="c", bufs=1) as cp,
            tc.tile_pool(name="ps", bufs=2, space="PSUM") as pp,
        ):
            hts_sb = cp.tile([F, DP], BF16)
            nc.sync.dma_start(hts_sb[:], hTs[:])
            htow_sb = cp.tile([F, DP], BF16)
            nc.sync.dma_start(htow_sb[:], hTow[:])
            waug_sb = cp.tile([F, 72], BF16)
            nc.sync.dma_start(waug_sb[:], Waug[:])
            skipw_sb = cp.tile([F, 64], BF16)
            nc.sync.dma_start(skipw_sb[:], skipW[:])
            bias_sb = cp.tile([P, 64], F32)
            nc.sync.dma_start(bias_sb[:], biasR[:])

            tstage = cp.tile([P, NWIN, P], BF16)
            asb = cp.tile([P, NWIN, 4], F32)
            ssb = cp.tile([P, NWIN, 64], F32)
            nc.gpsimd.memset(tstage[:], 0)
            for i in range(NWIN):
                dps = pp.tile([P, 72], F32, space="PSUM", tag="dps")
                nc.tensor.matmul(dps[:], hts_sb[:, i * P:(i + 1) * P],
                                 waug_sb[:], start=True, stop=True)
                nc.vector.tensor_copy(tstage[:, i, 0:64], dps[:, 0:64])
                tf32 = tstage[:].bitcast(F32)
                nc.vector.tensor_copy(tf32[:, i, 32:36], dps[:, 64:68])
                aps = pp.tile([P, 4], F32, space="PSUM", tag="aps")
                nc.tensor.matmul(aps[:], htow_sb[:, i * P:(i + 1) * P],
                                 waug_sb[:, 68:72], start=True, stop=True)
                sps = pp.tile([P, 64], F32, space="PSUM", tag="sps")
                nc.tensor.matmul(sps[:], htow_sb[:, i * P:(i + 1) * P],
                                 skipw_sb[:], start=True, stop=True)
                nc.vector.tensor_copy(asb[:, i, :], aps[:])
                nc.vector.tensor_tensor(ssb[:, i, :], sps[:],
                                        bias_sb[:], AluOpType.add)
            nc.sync.dma_start(
                tshard[:].rearrange("(p i) w -> p i w", p=P), tstage[:])
            nc.sync.dma_start(
                aldv[:].rearrange("(p i) h -> p i h", p=P), asb[:])
            nc.sync.dma_start(
                skipd[:].rearrange("(p i) c -> p i c", p=P), ssb[:])
    nc.compile()
    return nc


# ------------------------------------------------------------- launch B (edge)

def build_edge(shared, l2):
    chunks = shared["chunks"]
    nchunks = shared["nchunks"]
    nlo = shared["nlo"]
    SLO, SHI = shared["SLO"], shared["SHI"]
    GC = GC2 if l2 else GC01
    NW = 260 if l2 else 68

    nc = bacc.Bacc("TRN2", target_bir_lowering=False, debug=False,
                   num_swdge_queues=NQ)
    table = nc.dram_tensor("table", [TROWS, P], BF16, kind="ExternalInput")
    idx_lo = nc.dram_tensor("idx_lo", [P, max(SLO, 16) // 16], I16,
                            kind="ExternalInput")
    idx_hi = nc.dram_tensor("idx_hi", [P, max(SHI, 16) // 16], I16,
                            kind="ExternalInput")
    dstloc = nc.dram_tensor("dstloc", [P, nchunks], BF16,
                            kind="ExternalInput")
    alde_in = nc.dram_tensor("alde", [P, nchunks, 4], F32,
                             kind="ExternalInput")
    skipd_in = nc.dram_tensor("skipd", [DP, 64], F32, kind="ExternalInput")
    iota_in = nc.dram_tensor("iota", [P, P], BF16, kind="ExternalInput")
    if l2:
        w2_in = nc.dram_tensor("w2", [P, 2, 64], BF16, kind="ExternalInput")
        ident_in = nc.dram_tensor("ident", [P, P], BF16, kind="ExternalInput")
    y_out = nc.dram_tensor("y", [DP, 64], F32, kind="ExternalOutput")

    # group schedule: runs of <= GC chunks, same half
    groups = []
    k = 0
    while k < nchunks:
        k1 = min(k + GC, nchunks, nlo if k < nlo else nchunks)
        groups.append((k, k1))
        k = k1
    first = [False] * nchunks
    last = [False] * nchunks
    wlast = [False] * nchunks
    seen = set()
    wl = {}
    for k, (w, h) in enumerate(chunks):
        if (h, w) not in seen:
            seen.add((h, w))
            first[k] = True
        if k + 1 >= nchunks or chunks[k + 1] != (w, h):
            last[k] = True
        wl[w] = k
    for w, k in wl.items():
        wlast[k] = True

    with tile.TileContext(nc) as tc:
        with (
            tc.tile_pool(name="c", bufs=1) as cp,
            tc.tile_pool(name="g", bufs=3) as gp,
            tc.tile_pool(name="s", bufs=3) as sp,
            tc.tile_pool(name="pw", bufs=4, space="PSUM") as pw,
            tc.tile_pool(name="pt", bufs=2, space="PSUM") as pt,
        ):
            dloc_sb = cp.tile([P, nchunks], BF16)
            nc.sync.dma_start(dloc_sb[:], dstloc[:])
            alde_sb = cp.tile([P, nchunks, 4], F32)
            nc.sync.dma_start(alde_sb[:], alde_in[:])
            skipd_sb = cp.tile([P, NWIN, 64], F32)
            nc.sync.dma_start(
                skipd_sb[:], skipd_in[:].rearrange("(i p) c -> p i c", p=P))
            iota_sb = cp.tile([P, 1, P], BF16)
            nc.sync.dma_start(iota_sb[:, 0, :], iota_in[:])
            if l2:
                w2_sb = cp.tile([P, 2, 64], BF16)
                nc.sync.dma_start(w2_sb[:], w2_in[:])
                ident_sb = cp.tile([P, P], BF16)
                nc.sync.dma_start(ident_sb[:], ident_in[:])
            msum = cp.tile([P, NWIN, NW], F32)
            nc.gpsimd.memset(msum[:], 0)
            y_sb = cp.tile([P, NWIN, 64], F32)

            win_ps = {}
            for gi, (k0, k1) in enumerate(groups):
                T = k1 - k0
                h = chunks[k0][1]
                base = table[0: LO_ROWS + 1, :] if h == 0 \
                    else table[LO_ROWS + 1: TROWS, :]
                o16 = (k0 * P if h == 0 else (k0 - nlo) * P) // 16
                idx_t = sp.tile([P, GC * 8], I16, tag="idx")
                nc.sync.dma_start(
                    idx_t[:, : T * 8],
                    (idx_lo if h == 0 else idx_hi)[:, o16: o16 + T * 8])
                gt = gp.tile([P, GC, P], BF16, tag="g")
                nc.gpsimd.dma_gather(
                    gt[:, :T, :], base, idx_t[:, : T * 8], T * P, T * P, P,
                    single_packet=False, queue_num=gi % NQ)

                zf = sp.tile([P, GC, 4], F32, tag="z")
                gf = gt[:].bitcast(F32)
                nc.vector.tensor_tensor(zf[:, :T, :], gf[:, :T, 32:36],
                                        alde_sb[:, k0:k1, :], AluOpType.add)
                nc.vector.scalar_tensor_tensor(
                    zf[:, :T, :], zf[:, :T, :], 0.2, zf[:, :T, :],
                    AluOpType.mult, AluOpType.max)
                sel = sp.tile([P, GC, P], BF16, tag="sel")
                nc.vector.tensor_tensor(
                    sel[:, :T, :],
                    iota_sb[:].to_broadcast([P, T, P]),
                    dloc_sb[:, k0:k1, None].to_broadcast([P, T, P]),
                    AluOpType.is_equal)

                if l2:
                    rhs = gp.tile([P, GC, 260], BF16, tag="rhs")
                    nc.scalar.activation(rhs[:, :T, 256:260], zf[:, :T, :],
                                         mybir.ActivationFunctionType.Exp)
                    nc.vector.tensor_tensor(
                        rhs[:, :T, 0:256].rearrange(
                            "p t (h c) -> p t h c", c=64),
                        gt[:, :T, None, 0:64].to_broadcast([P, T, 4, 64]),
                        rhs[:, :T, 256:260, None].to_broadcast([P, T, 4, 64]),
                        AluOpType.mult)
                else:
                    nc.scalar.activation(gt[:, :T, 64:68], zf[:, :T, :],
                                         mybir.ActivationFunctionType.Exp)
                    nc.vector.tensor_tensor(
                        gt[:, :T, 0:64].rearrange("p t (h c) -> p t h c", c=16),
                        gt[:, :T, 0:64].rearrange("p t (h c) -> p t h c", c=16),
                        gt[:, :T, 64:68, None].to_broadcast([P, T, 4, 16]),
                        AluOpType.mult)

                for t in range(T):
                    k = k0 + t
                    w, hh = chunks[k]
                    if first[k]:
                        win_ps[w] = pw.tile([P, NW], F32, space="PSUM",
                                            tag="win", name=f"win{w}h{hh}")
                    rhs_ap = rhs[:, t, :] if l2 else gt[:, t, 0:68]
                    nc.tensor.matmul(win_ps[w][:], sel[:, t, :], rhs_ap,
                                     start=first[k], stop=last[k],
                                     skip_group_check=True)
                    if last[k]:
                        pwin = win_ps.pop(w)
                        nc.vector.tensor_tensor(msum[:, w, :], msum[:, w, :],
                                                pwin[:], AluOpType.add)
                    if l2 and wlast[k]:
                        # per-window W2 drain, overlapped with later groups
                        recw = sp.tile([P, 4], F32, tag="recw")
                        nc.vector.reciprocal(recw[:], msum[:, w, 256:260])
                        snw = sp.tile([P, 4, 64], BF16, tag="snw")
                        nc.vector.tensor_tensor(
                            snw[:],
                            msum[:, w, 0:256].rearrange(
                                "p (h c) -> p h c", c=64),
                            recw[:, :, None].to_broadcast([P, 4, 64]),
                            AluOpType.mult)
                        yps = pt.tile([P, 64], F32, space="PSUM", tag="yps")
                        for j in range(2):
                            tp = pt.tile([P, P], BF16, space="PSUM", tag="tp")
                            nc.tensor.matmul(
                                tp[:],
                                snw[:].rearrange("p h c -> p (h c)")
                                      [:, j * P:(j + 1) * P],
                                ident_sb[:], is_transpose=True,
                                start=True, stop=True, skip_group_check=True)
                            st = sp.tile([P, P], BF16, tag="st")
                            nc.scalar.activation(
                                st[:], tp[:],
                                mybir.ActivationFunctionType.Copy)
                            nc.tensor.matmul(yps[:], st[:], w2_sb[:, j, :],
                                             start=(j == 0), stop=(j == 1),
                                             skip_group_check=True)
                        nc.vector.scalar_tensor_tensor(
                            y_sb[:, w, :], yps[:], 0.25, skipd_sb[:, w, :],
                            AluOpType.mult, AluOpType.add)

            # ---------------- batched normalize + output
            if not l2:
                rec = cp.tile([P, NWIN, 4], F32)
                nc.vector.reciprocal(rec[:], msum[:, :, NW - 4: NW])
                nc.vector.tensor_tensor(
                    y_sb[:].rearrange("p w (h c) -> p w h c", c=16),
                    msum[:, :, 0:64].rearrange("p w (h c) -> p w h c", c=16),
                    rec[:, :, :, None].to_broadcast([P, NWIN, 4, 16]),
                    AluOpType.mult)
                nc.vector.tensor_tensor(y_sb[:], y_sb[:], skipd_sb[:],
                                        AluOpType.add)
                nc.vector.tensor_scalar_max(y_sb[:], y_sb[:], 0.0)
            nc.sync.dma_start(
                y_out[:].rearrange("(i p) c -> p i c", p=P), y_sb[:])
    nc.compile()
    return nc


# ------------------------------------------------------------------ driver

_CACHE = {}
_DBG = []
_EXEC_NS = []
_RESULTS = []


def _blockdiag(a):
    H, C = a.shape
    m = np.zeros((H * C, H), np.float32)
    for hh in range(H):
        m[hh * C: (hh + 1) * C, hh] = a[hh]
    return m


def _bf(x):
    return np.ascontiguousarray(np.asarray(x, np.float32)
                                .astype(ml_dtypes.bfloat16))


def kernel(**inp):
    x = np.asarray(inp["x"], np.float32)
    ei = np.asarray(inp["edge_index"], np.int64)
    N, IN = x.shape
    E = ei.shape[1]

    loops = np.arange(N, dtype=np.int64)
    src = np.concatenate([ei[0], loops])
    dst = np.concatenate([ei[1], loops])

    pkey = ("plan", N, E, hash(ei.tobytes()))
    if pkey not in _CACHE:
        _CACHE[pkey] = build_plan(src, dst, N)
    shared, plans = _CACHE[pkey]

    def prep01(Wv, a_s, a_d, cb, sW, sb, g, b, m, v):
        Wv, sW = np.asarray(Wv, np.float32), np.asarray(sW, np.float32)
        bns = (np.asarray(g) / np.sqrt(np.asarray(v) + EPS)).astype(np.float32)
        bnt = (np.asarray(b) - np.asarray(m) * bns).astype(np.float32)
        Waug = np.concatenate(
            [Wv * bns[None, :], Wv @ _blockdiag(np.asarray(a_s)),
             Wv @ _blockdiag(np.asarray(a_d))], 1)
        return (Waug, sW * bns[None, :],
                (np.asarray(cb) + np.asarray(sb)) * bns + bnt, None)

    def prep2(Wv, a_s, a_d, cb, sW, sb):
        Wv = np.asarray(Wv, np.float32)
        Waug = np.concatenate(
            [np.eye(64, dtype=np.float32), Wv @ _blockdiag(np.asarray(a_s)),
             Wv @ _blockdiag(np.asarray(a_d))], 1)
        w2 = np.ascontiguousarray(
            Wv.reshape(64, 4, 64).transpose(1, 0, 2).reshape(256, 64)
            .reshape(2, 128, 64).transpose(1, 0, 2))
        return (Waug, np.asarray(sW, np.float32),
                np.asarray(cb) + np.asarray(sb), w2)

    Ls = [
        prep01(inp["conv0_W"], inp["conv0_as"], inp["conv0_ad"],
               inp["conv0_b"], inp["skip0_W"], inp["skip0_b"],
               inp["bn0_g"], inp["bn0_b"], inp["bn0_m"], inp["bn0_v"]),
        prep01(inp["conv1_W"], inp["conv1_as"], inp["conv1_ad"],
               inp["conv1_b"], inp["skip1_W"], inp["skip1_b"],
               inp["bn1_g"], inp["bn1_b"], inp["bn1_m"], inp["bn1_v"]),
        prep2(inp["conv2_W"], inp["conv2_as"], inp["conv2_ad"],
              inp["conv2_b"], inp["skip2_W"], inp["skip2_b"]),
    ]

    iota_np = np.tile(np.arange(P, dtype=np.float32), (P, 1)).astype(
        ml_dtypes.bfloat16)
    ident_np = np.eye(P, dtype=np.float32).astype(ml_dtypes.bfloat16)
    # sentinel row: zero feats, al_src = -40 (f32 packed in bf16 slots 64..71)
    sent_view = np.zeros(P, np.uint16)
    sent_view[64:72] = np.full(4, SENT_ALS, np.float32).view(np.uint16)
    sent = sent_view.view(ml_dtypes.bfloat16)

    h = x
    for li in range(3):
        F = IN if li == 0 else 64
        l2 = li == 2
        Waug, skipWf, biasv, w2 = Ls[li]
        akey = ("A", F)
        if akey not in _CACHE:
            _CACHE[akey] = build_dense(F)
        bkey = ("B", l2)
        if bkey not in _CACHE:
            _CACHE[bkey] = build_edge(shared, l2)

        hT = h.T.astype(np.float32)
        base_a = {
            "Waug": _bf(Waug),
            "skipW": _bf(skipWf),
            "biasR": np.tile(np.asarray(biasv, np.float32), (P, 1)),
        }
        a_maps = []
        for c in range(NC):
            node = c * SB + PERMC
            valid_s = PERMC < SB
            hts = np.zeros((F, DP), np.float32)
            hts[:, valid_s] = hT[:, node[valid_s]]
            vm = plans[c]["vmap"][PERMC]
            valid_d = vm >= 0
            htow = np.zeros((F, DP), np.float32)
            htow[:, valid_d] = hT[:, vm[valid_d]]
            a_maps.append(dict(base_a, hTs=_bf(hts), hTow=_bf(htow)))
        res_a = run_bass_kernel_spmd(_CACHE[akey], a_maps,
                                     core_ids=list(range(NC)))
        _RESULTS.append(res_a)
        if res_a.exec_time_ns:
            _EXEC_NS.append(res_a.exec_time_ns)

        tbl = np.empty((TROWS, P), ml_dtypes.bfloat16)
        for c in range(4):
            tbl[c * RB:(c + 1) * RB] = res_a.results[c]["tshard"]
        tbl[LO_ROWS] = sent
        for c in range(4, 8):
            tbl[LO_ROWS + 1 + (c - 4) * RB: LO_ROWS + 1 + (c - 3) * RB] = \
                res_a.results[c]["tshard"]
        tbl[TROWS - 1] = sent

        base_b = {"table": tbl, "iota": iota_np}
        if l2:
            base_b["w2"] = _bf(w2)
            base_b["ident"] = ident_np
        b_maps = []
        for c in range(NC):
            aldv = res_a.results[c]["aldv"]
            alde = aldv[plans[c]["slot_dst"]]  # [128, nchunks, 4]
            b_maps.append(dict(
                base_b,
                idx_lo=plans[c]["idx_lo"], idx_hi=plans[c]["idx_hi"],
                dstloc=plans[c]["dstloc"],
                alde=np.ascontiguousarray(alde.astype(np.float32)),
                skipd=np.ascontiguousarray(
                    res_a.results[c]["skipd"].astype(np.float32))))
        res_b = run_bass_kernel_spmd(_CACHE[bkey], b_maps,
                                     core_ids=list(range(NC)))
        _RESULTS.append(res_b)
        if res_b.exec_time_ns:
            _EXEC_NS.append(res_b.exec_time_ns)

        hn = np.zeros((N, 64), np.float32)
        for c in range(NC):
            vm = plans[c]["vmap"]
            valid = vm >= 0
            hn[vm[valid]] = res_b.results[c]["y"][valid]
        h = hn
        _DBG.append(h)
    return h


# revision 28
# speedup vs baseline: 3.8253x; 1.0412x over previous
"""GAT (3-layer, PyG-style) Trainium2 Bass kernel, 8-core dst-sharded. v2.

Self-contained: takes full inputs, shards internally, returns full output.

Design (per layer -> two SPMD launches):
  Launch A (dense, src-sharded): each core computes its 1/8 of the gather
    table rows [64 feat bf16 | 4 al_src f32] = 256B/row via PE matmuls with
    Waug = [W(*bns) | W@blk(a_src) | W@blk(a_dst)], plus per-dst al_dst and
    the dense skip+bias rows for its dst shard. Host assembles the full
    table (plus 2 sentinel rows) and expands the per-slot al_dst stream.
  Launch B (edge, dst-sharded): exact-CSR edge slots (chunks of 128 slots,
    one (window, src-half) per chunk; schedule shared across cores, per-core
    counts balanced by an LPT assignment of dst nodes to (core, window)).
    Per group of chunks: dma_gather of 256B rows (4 SWDGE queues round-
    robin), ex = exp(leaky_relu(al_s + al_d)) on DVE/Act, messages scaled
    by ex, segment-sum via per-chunk one-hot matmuls (sel built on DVE by
    comparing a shipped dstloc column against an iota row) accumulating
    [msg | sum_ex] per 128-dst window in PSUM, drained into an SBUF
    accumulator. One batched normalize pass at the end.
  Layer 2 uses GAT linearity: segment-sum runs in h1-space (same 256B rows)
    with 4 per-head ex-scaled copies; W2 is applied per window afterwards
    (PE transpose + 2 matmuls), then mean over heads.
"""
import numpy as np
import ml_dtypes

import concourse.bacc as bacc
import concourse.mybir as mybir
import concourse.tile as tile
from concourse.alu_op_type import AluOpType
from concourse.bass_utils import run_bass_kernel_spmd

BF16 = mybir.dt.bfloat16
F32 = mybir.dt.float32
I16 = mybir.dt.int16

NC = 8
P = 128
EPS = 1e-5
SENT_ALS = -40.0
NQ = 4          # SWDGE queues
GC01 = 64       # chunks per gather group, layers 0/1
GC2 = 32        # chunks per gather group, layer 2

N_NODES = 50000
SB = N_NODES // NC          # src nodes per core (6250)
NWIN = 51                   # dst windows per core
WCAP = 125                  # dsts per window (< 128 for ceil slack)
DP = NWIN * P               # padded dst slots per core (6528)
RB = DP                     # table row block per core (6272)
LO_ROWS = 4 * RB            # rows in the lo half (25088)
SENT_REL = LO_ROWS          # sentinel index relative to each half base
TROWS = 2 * (LO_ROWS + 1)


def _ceil(a, b):
    return -(-a // b)


# ----------------------------------------------------------------- planning

def build_plan(src, dst, N):
    assert N == N_NODES
    # src -> permuted table row (relative to half base) and half flag
    s_core = src // SB
    arow = (src % SB) + s_core * RB
    half = (s_core >= 4).astype(np.int64)
    rel = np.where(half == 0, arow, arow - 4 * RB)

    # balanced assignment of dst nodes to (core, window) buckets, keeping
    # BOTH halves' per-bucket loads even (ceil(max_c/128) drives slot count)
    deg = np.zeros((N, 2), np.int64)
    np.add.at(deg, (dst, half), 1)
    tot = deg.sum(1)
    order = np.argsort(-tot, kind="stable")
    NB = NC * NWIN
    LO = np.zeros(NB, np.float64)
    HI = np.zeros(NB, np.float64)
    bcount = np.zeros(NB, np.int64)
    BIG = 1e18
    dst_c = np.empty(N, np.int64)
    dst_w = np.empty(N, np.int64)
    dst_p = np.empty(N, np.int64)
    for n in order:
        score = np.maximum(LO + deg[n, 0], HI + deg[n, 1])
        b = int(np.argmin(score))
        dst_c[n] = b // NWIN
        dst_w[n] = b % NWIN
        dst_p[n] = bcount[b]
        bcount[b] += 1
        LO[b] += deg[n, 0]
        HI[b] += deg[n, 1]
        if bcount[b] >= WCAP:
            LO[b] = HI[b] = BIG

    vmap = np.full((NC, DP), -1, np.int64)
    vmap[dst_c, dst_w * P + dst_p] = np.arange(N)

    # per (core, window, half) edge counts -> shared chunk schedule
    ec = dst_c[dst]
    ew = dst_w[dst]
    epos = dst_p[dst]
    cnt = np.zeros((NC, NWIN, 2), np.int64)
    np.add.at(cnt, (ec, ew, half), 1)
    nch = np.zeros((NWIN, 2), np.int64)
    for w in range(NWIN):
        for h in range(2):
            nch[w, h] = _ceil(int(cnt[:, w, h].max()), P) if cnt[:, w, h].max() else 0
        if nch[w].sum() == 0:
            nch[w, 0] = 1  # keep every window on the schedule
    chunks = []          # (w, h) in schedule order: lo sweep then hi sweep
    for h in range(2):
        for w in range(NWIN):
            chunks += [(w, h)] * int(nch[w, h])
    nchunks = len(chunks)
    slot_base = {}       # (w, h) -> first slot index in the combined stream
    o = 0
    for (w, h) in chunks:
        slot_base.setdefault((w, h), o * P)
        o += 1
    nlo = int(nch[:, 0].sum())
    SLO, SHI = nlo * P, (nchunks - nlo) * P

    # per-core streams
    plans = []
    for c in range(NC):
        em = ec == c
        eh = half[em]
        key = eh * (NWIN * P) + ew[em] * P  # per (h, w) group key base
        # stable ordering by (h, w); position within group via argsort
        si = np.argsort(key, kind="stable")
        erel = rel[em][si]
        ewk = ew[em][si]
        ehk = eh[si]
        epk = epos[em][si]
        idx_all = np.full(nchunks * P, SENT_REL, np.int64)
        dloc = np.full(nchunks * P, -1.0, np.float32)
        sdst = np.zeros(nchunks * P, np.int64)
        # group runs: edges sorted by (h, w); fill each group's slot range
        bounds = np.searchsorted(
            ehk * NWIN + ewk, np.arange(2 * NWIN + 1))
        for h in range(2):
            for w in range(NWIN):
                a, b = bounds[h * NWIN + w], bounds[h * NWIN + w + 1]
                if a == b:
                    continue
                s0 = slot_base[(w, h)]
                m = b - a
                idx_all[s0: s0 + m] = erel[a:b]
                dloc[s0: s0 + m] = epk[a:b]
                sdst[s0: s0 + m] = w * P + epk[a:b]
        idx_lo = _wrap16(idx_all[:SLO])
        idx_hi = _wrap16(idx_all[SLO:])
        dstloc = dloc.reshape(nchunks, P).T.astype(ml_dtypes.bfloat16)
        sdst_w = sdst.reshape(nchunks, P).transpose(1, 0)
        plans.append(dict(vmap=vmap[c], idx_lo=idx_lo, idx_hi=idx_hi,
                          dstloc=np.ascontiguousarray(dstloc),
                          slot_dst=np.ascontiguousarray(sdst_w)))
    shared = dict(chunks=chunks, nchunks=nchunks, nlo=nlo, SLO=SLO, SHI=SHI)
    return shared, plans


def _wrap16(stream):
    S = len(stream)
    w = stream.reshape(S // 16, 16).T.astype(np.int16)
    return np.ascontiguousarray(np.tile(w, (8, 1)))


PERMC = (np.arange(DP) % P) * NWIN + np.arange(DP) // P  # col k -> slot p*49+i


# ------------------------------------------------------------- launch A (dense)

def build_dense(F):
    nc = bacc.Bacc("TRN2", target_bir_lowering=False, debug=False)
    hTs = nc.dram_tensor("hTs", [F, DP], BF16, kind="ExternalInput")
    hTow = nc.dram_tensor("hTow", [F, DP], BF16, kind="ExternalInput")
    Waug = nc.dram_tensor("Waug", [F, 72], BF16, kind="ExternalInput")
    skipW = nc.dram_tensor("skipW", [F, 64], BF16, kind="ExternalInput")
    biasR = nc.dram_tensor("biasR", [P, 64], F32, kind="ExternalInput")
    tshard = nc.dram_tensor("tshard", [DP, P], BF16, kind="ExternalOutput")
    aldv = nc.dram_tensor("aldv", [DP, 4], F32, kind="ExternalOutput")
    skipd = nc.dram_tensor("skipd", [DP, 64], F32, kind="ExternalOutput")
    featd = nc.dram_tensor("featd", [DP, 64], BF16, kind="ExternalOutput")
    selfz = nc.dram_tensor("selfz", [DP, 4], F32, kind="ExternalOutput")

    with tile.TileContext(nc) as tc:
        with (
            tc.tile_pool(name="c", bufs=1) as cp,
            tc.tile_pool(name="ps", bufs=2, space="PSUM") as pp,
        ):
            hts_sb = cp.tile([F, DP], BF16)
            nc.sync.dma_start(hts_sb[:], hTs[:])
            htow_sb = cp.tile([F, DP], BF16)
            nc.sync.dma_start(htow_sb[:], hTow[:])
            waug_sb = cp.tile([F, 72], BF16)
            nc.sync.dma_start(waug_sb[:], Waug[:])
            skipw_sb = cp.tile([F, 64], BF16)
            nc.sync.dma_start(skipw_sb[:], skipW[:])
            bias_sb = cp.tile([P, 64], F32)
            nc.sync.dma_start(bias_sb[:], biasR[:])

            tstage = cp.tile([P, NWIN, P], BF16)
            asb = cp.tile([P, NWIN, 4], F32)
            ssb = cp.tile([P, NWIN, 64], F32)
            fsb = cp.tile([P, NWIN, 64], BF16)
            szb = cp.tile([P, NWIN, 4], F32)
            nc.gpsimd.memset(tstage[:], 0)
            for i in range(NWIN):
                dps = pp.tile([P, 72], F32, space="PSUM", tag="dps")
                nc.tensor.matmul(dps[:], hts_sb[:, i * P:(i + 1) * P],
                                 waug_sb[:], start=True, stop=True)
                nc.vector.tensor_copy(tstage[:, i, 0:64], dps[:, 0:64])
                tf32 = tstage[:].bitcast(F32)
                nc.vector.tensor_copy(tf32[:, i, 32:36], dps[:, 64:68])
                ops_ = pp.tile([P, 72], F32, space="PSUM", tag="ops")
                nc.tensor.matmul(ops_[:], htow_sb[:, i * P:(i + 1) * P],
                                 waug_sb[:], start=True, stop=True)
                sps = pp.tile([P, 64], F32, space="PSUM", tag="sps")
                nc.tensor.matmul(sps[:], htow_sb[:, i * P:(i + 1) * P],
                                 skipw_sb[:], start=True, stop=True)
                nc.vector.tensor_copy(asb[:, i, :], ops_[:, 68:72])
                nc.vector.tensor_copy(fsb[:, i, :], ops_[:, 0:64])
                nc.vector.tensor_tensor(szb[:, i, :], ops_[:, 64:68],
                                        asb[:, i, :], AluOpType.add)
                nc.vector.tensor_tensor(ssb[:, i, :], sps[:],
                                        bias_sb[:], AluOpType.add)
            nc.sync.dma_start(
                tshard[:].rearrange("(p i) w -> p i w", p=P), tstage[:])
            nc.sync.dma_start(
                aldv[:].rearrange("(p i) h -> p i h", p=P), asb[:])
            nc.sync.dma_start(
                skipd[:].rearrange("(p i) c -> p i c", p=P), ssb[:])
            nc.sync.dma_start(
                featd[:].rearrange("(p i) c -> p i c", p=P), fsb[:])
            nc.sync.dma_start(
                selfz[:].rearrange("(p i) h -> p i h", p=P), szb[:])
    nc.compile()
    return nc


# ------------------------------------------------------------- launch B (edge)

def build_edge(shared, l2):
    chunks = shared["chunks"]
    nchunks = shared["nchunks"]
    nlo = shared["nlo"]
    SLO, SHI = shared["SLO"], shared["SHI"]
    GC = GC2 if l2 else GC01
    NW = 260 if l2 else 68

    nc = bacc.Bacc("TRN2", target_bir_lowering=False, debug=False,
                   num_swdge_queues=NQ)
    table = nc.dram_tensor("table", [TROWS, P], BF16, kind="ExternalInput")
    idx_lo = nc.dram_tensor("idx_lo", [P, max(SLO, 16) // 16], I16,
                            kind="ExternalInput")
    idx_hi = nc.dram_tensor("idx_hi", [P, max(SHI, 16) // 16], I16,
                            kind="ExternalInput")
    dstloc = nc.dram_tensor("dstloc", [P, nchunks], BF16,
                            kind="ExternalInput")
    alde_in = nc.dram_tensor("alde", [P, nchunks, 4], F32,
                             kind="ExternalInput")
    skipd_in = nc.dram_tensor("skipd", [DP, 64], F32, kind="ExternalInput")
    featd_in = nc.dram_tensor("featd", [DP, 64], BF16, kind="ExternalInput")
    selfz_in = nc.dram_tensor("selfz", [DP, 4], F32, kind="ExternalInput")
    iota_in = nc.dram_tensor("iota", [P, P], BF16, kind="ExternalInput")
    if l2:
        w2_in = nc.dram_tensor("w2", [P, 2, 64], BF16, kind="ExternalInput")
        ident_in = nc.dram_tensor("ident", [P, P], BF16, kind="ExternalInput")
    y_out = nc.dram_tensor("y", [DP, 64], F32, kind="ExternalOutput")

    # group schedule: runs of <= GC chunks, same half
    groups = []
    k = 0
    while k < nchunks:
        k1 = min(k + GC, nchunks, nlo if k < nlo else nchunks)
        groups.append((k, k1))
        k = k1
    first = [False] * nchunks
    last = [False] * nchunks
    wlast = [False] * nchunks
    seen = set()
    wl = {}
    for k, (w, h) in enumerate(chunks):
        if (h, w) not in seen:
            seen.add((h, w))
            first[k] = True
        if k + 1 >= nchunks or chunks[k + 1] != (w, h):
            last[k] = True
        wl[w] = k
    for w, k in wl.items():
        wlast[k] = True

    with tile.TileContext(nc) as tc:
        with (
            tc.tile_pool(name="c", bufs=1) as cp,
            tc.tile_pool(name="g", bufs=3 if l2 else 4) as gp,
            tc.tile_pool(name="r", bufs=2) as rp,
            tc.tile_pool(name="s", bufs=3) as sp,
            tc.tile_pool(name="pw", bufs=4, space="PSUM") as pw,
            tc.tile_pool(name="pt", bufs=2, space="PSUM") as pt,
        ):
            # big constant loads go on the Act engine's DGE so the first
            # gather groups' idx loads (sync engine) aren't queued behind them
            dloc_sb = cp.tile([P, nchunks], BF16)
            nc.scalar.dma_start(dloc_sb[:], dstloc[:])
            alde_sb = cp.tile([P, nchunks, 4], F32)
            nc.scalar.dma_start(alde_sb[:], alde_in[:])
            skipd_sb = cp.tile([P, NWIN, 64], F32)
            nc.scalar.dma_start(
                skipd_sb[:], skipd_in[:].rearrange("(i p) c -> p i c", p=P))
            featd_sb = cp.tile([P, NWIN, 64], BF16)
            nc.scalar.dma_start(
                featd_sb[:], featd_in[:].rearrange("(i p) c -> p i c", p=P))
            selfz_sb = cp.tile([P, NWIN, 4], F32)
            nc.scalar.dma_start(
                selfz_sb[:], selfz_in[:].rearrange("(i p) h -> p i h", p=P))
            iota_sb = cp.tile([P, 1, P], BF16)
            nc.scalar.dma_start(iota_sb[:, 0, :], iota_in[:])
            if l2:
                w2_sb = cp.tile([P, 2, 64], BF16)
                nc.scalar.dma_start(w2_sb[:], w2_in[:])
                ident_sb = cp.tile([P, P], BF16)
                nc.scalar.dma_start(ident_sb[:], ident_in[:])
            msum = cp.tile([P, NWIN, NW], F32)
            y_sb = cp.tile([P, NWIN, 64], F32)

            # init msum with the dense self-loop contributions
            exself = cp.tile([P, NWIN, 4], F32)
            nc.vector.scalar_tensor_tensor(
                exself[:], selfz_sb[:], 0.2, selfz_sb[:],
                AluOpType.mult, AluOpType.max)
            nc.scalar.activation(exself[:], exself[:],
                                 mybir.ActivationFunctionType.Exp)
            nc.vector.tensor_copy(msum[:, :, NW - 4: NW], exself[:])
            if l2:
                nc.vector.tensor_tensor(
                    msum[:, :, 0:256].rearrange("p w (h c) -> p w h c", c=64),
                    featd_sb[:, :, None, :].to_broadcast([P, NWIN, 4, 64]),
                    exself[:, :, :, None].to_broadcast([P, NWIN, 4, 64]),
                    AluOpType.mult)
            else:
                nc.vector.tensor_tensor(
                    msum[:, :, 0:64].rearrange("p w (h c) -> p w h c", c=16),
                    featd_sb[:].rearrange("p w (h c) -> p w h c", c=16),
                    exself[:, :, :, None].to_broadcast([P, NWIN, 4, 16]),
                    AluOpType.mult)

            win_ps = {}
            for gi, (k0, k1) in enumerate(groups):
                T = k1 - k0
                h = chunks[k0][1]
                base = table[0: LO_ROWS + 1, :] if h == 0 \
                    else table[LO_ROWS + 1: TROWS, :]
                o16 = (k0 * P if h == 0 else (k0 - nlo) * P) // 16
                idx_t = sp.tile([P, GC * 8], I16, tag="idx")
                nc.sync.dma_start(
                    idx_t[:, : T * 8],
                    (idx_lo if h == 0 else idx_hi)[:, o16: o16 + T * 8])
                gt = gp.tile([P, GC, P], BF16, tag="g")
                nc.gpsimd.dma_gather(
                    gt[:, :T, :], base, idx_t[:, : T * 8], T * P, T * P, P,
                    single_packet=False, queue_num=gi % NQ)

                zf = sp.tile([P, GC, 4], F32, tag="z")
                gf = gt[:].bitcast(F32)
                nc.vector.tensor_tensor(zf[:, :T, :], gf[:, :T, 32:36],
                                        alde_sb[:, k0:k1, :], AluOpType.add)
                nc.vector.scalar_tensor_tensor(
                    zf[:, :T, :], zf[:, :T, :], 0.2, zf[:, :T, :],
                    AluOpType.mult, AluOpType.max)
                sel = sp.tile([P, GC, P], BF16, tag="sel")
                nc.vector.tensor_tensor(
                    sel[:, :T, :],
                    iota_sb[:].to_broadcast([P, T, P]),
                    dloc_sb[:, k0:k1, None].to_broadcast([P, T, P]),
                    AluOpType.is_equal)

                if l2:
                    rhs = rp.tile([P, GC, 260], BF16, tag="rhs")
                    nc.scalar.activation(rhs[:, :T, 256:260], zf[:, :T, :],
                                         mybir.ActivationFunctionType.Exp)
                    nc.vector.tensor_tensor(
                        rhs[:, :T, 0:256].rearrange(
                            "p t (h c) -> p t h c", c=64),
                        gt[:, :T, None, 0:64].to_broadcast([P, T, 4, 64]),
                        rhs[:, :T, 256:260, None].to_broadcast([P, T, 4, 64]),
                        AluOpType.mult)
                else:
                    nc.scalar.activation(gt[:, :T, 64:68], zf[:, :T, :],
                                         mybir.ActivationFunctionType.Exp)
                    nc.vector.tensor_tensor(
                        gt[:, :T, 0:64].rearrange("p t (h c) -> p t h c", c=16),
                        gt[:, :T, 0:64].rearrange("p t (h c) -> p t h c", c=16),
                        gt[:, :T, 64:68, None].to_broadcast([P, T, 4, 16]),
                        AluOpType.mult)

                for t in range(T):
                    k = k0 + t
                    w, hh = chunks[k]
                    if first[k]:
                        win_ps[w] = pw.tile([P, NW], F32, space="PSUM",
                                            tag="win", name=f"win{w}h{hh}")
                    rhs_ap = rhs[:, t, :] if l2 else gt[:, t, 0:68]
                    nc.tensor.matmul(win_ps[w][:], sel[:, t, :], rhs_ap,
                                     start=first[k], stop=last[k],
                                     skip_group_check=True)
                    if last[k]:
                        pwin = win_ps.pop(w)
                        nc.vector.tensor_tensor(msum[:, w, :], msum[:, w, :],
                                                pwin[:], AluOpType.add)
                    if (not l2) and wlast[k]:
                        recw = sp.tile([P, 4], F32, tag="recw")
                        nc.vector.reciprocal(recw[:], msum[:, w, 64:68])
                        nc.vector.tensor_tensor(
                            y_sb[:, w, :].rearrange("p (h c) -> p h c", c=16),
                            msum[:, w, 0:64].rearrange("p (h c) -> p h c",
                                                       c=16),
                            recw[:, :, None].to_broadcast([P, 4, 16]),
                            AluOpType.mult)
                        nc.vector.tensor_tensor(y_sb[:, w, :], y_sb[:, w, :],
                                                skipd_sb[:, w, :],
                                                AluOpType.add)
                        nc.vector.tensor_scalar_max(y_sb[:, w, :],
                                                    y_sb[:, w, :], 0.0)
                    if l2 and wlast[k]:
                        # per-window W2 drain, overlapped with later groups
                        recw = sp.tile([P, 4], F32, tag="recw")
                        nc.vector.reciprocal(recw[:], msum[:, w, 256:260])
                        snw = sp.tile([P, 4, 64], BF16, tag="snw")
                        nc.vector.tensor_tensor(
                            snw[:],
                            msum[:, w, 0:256].rearrange(
                                "p (h c) -> p h c", c=64),
                            recw[:, :, None].to_broadcast([P, 4, 64]),
                            AluOpType.mult)
                        yps = pt.tile([P, 64], F32, space="PSUM", tag="yps")
                        for j in range(2):
                            tp = pt.tile([P, P], BF16, space="PSUM", tag="tp")
                            nc.tensor.matmul(
                                tp[:],
                                snw[:].rearrange("p h c -> p (h c)")
                                      [:, j * P:(j + 1) * P],
                                ident_sb[:], is_transpose=True,
                                start=True, stop=True, skip_group_check=True)
                            st = sp.tile([P, P], BF16, tag="st")
                            nc.scalar.activation(
                                st[:], tp[:],
                                mybir.ActivationFunctionType.Copy)
                            nc.tensor.matmul(yps[:], st[:], w2_sb[:, j, :],
                                             start=(j == 0), stop=(j == 1),
                                             skip_group_check=True)
                        nc.vector.scalar_tensor_tensor(
                            y_sb[:, w, :], yps[:], 0.25, skipd_sb[:, w, :],
                            AluOpType.mult, AluOpType.add)

            nc.sync.dma_start(
                y_out[:].rearrange("(i p) c -> p i c", p=P), y_sb[:])
    nc.compile()
    return nc


# ------------------------------------------------------------------ driver

_CACHE = {}
_DBG = []
_EXEC_NS = []
_RESULTS = []


def _blockdiag(a):
    H, C = a.shape
    m = np.zeros((H * C, H), np.float32)
    for hh in range(H):
        m[hh * C: (hh + 1) * C, hh] = a[hh]
    return m


def _bf(x):
    return np.ascontiguousarray(np.asarray(x, np.float32)
                                .astype(ml_dtypes.bfloat16))


def kernel(**inp):
    x = np.asarray(inp["x"], np.float32)
    ei = np.asarray(inp["edge_index"], np.int64)
    N, IN = x.shape
    E = ei.shape[1]

    # self-loops are handled densely in launch B; streams carry real edges
    src = ei[0]
    dst = ei[1]

    pkey = ("plan", N, E, hash(ei.tobytes()))
    if pkey not in _CACHE:
        _CACHE[pkey] = build_plan(src, dst, N)
    shared, plans = _CACHE[pkey]

    def prep01(Wv, a_s, a_d, cb, sW, sb, g, b, m, v):
        Wv, sW = np.asarray(Wv, np.float32), np.asarray(sW, np.float32)
        bns = (np.asarray(g) / np.sqrt(np.asarray(v) + EPS)).astype(np.float32)
        bnt = (np.asarray(b) - np.asarray(m) * bns).astype(np.float32)
        Waug = np.concatenate(
            [Wv * bns[None, :], Wv @ _blockdiag(np.asarray(a_s)),
             Wv @ _blockdiag(np.asarray(a_d))], 1)
        return (Waug, sW * bns[None, :],
                (np.asarray(cb) + np.asarray(sb)) * bns + bnt, None)

    def prep2(Wv, a_s, a_d, cb, sW, sb):
        Wv = np.asarray(Wv, np.float32)
        Waug = np.concatenate(
            [np.eye(64, dtype=np.float32), Wv @ _blockdiag(np.asarray(a_s)),
             Wv @ _blockdiag(np.asarray(a_d))], 1)
        w2 = np.ascontiguousarray(
            Wv.reshape(64, 4, 64).transpose(1, 0, 2).reshape(256, 64)
            .reshape(2, 128, 64).transpose(1, 0, 2))
        return (Waug, np.asarray(sW, np.float32),
                np.asarray(cb) + np.asarray(sb), w2)

    Ls = [
        prep01(inp["conv0_W"], inp["conv0_as"], inp["conv0_ad"],
               inp["conv0_b"], inp["skip0_W"], inp["skip0_b"],
               inp["bn0_g"], inp["bn0_b"], inp["bn0_m"], inp["bn0_v"]),
        prep01(inp["conv1_W"], inp["conv1_as"], inp["conv1_ad"],
               inp["conv1_b"], inp["skip1_W"], inp["skip1_b"],
               inp["bn1_g"], inp["bn1_b"], inp["bn1_m"], inp["bn1_v"]),
        prep2(inp["conv2_W"], inp["conv2_as"], inp["conv2_ad"],
              inp["conv2_b"], inp["skip2_W"], inp["skip2_b"]),
    ]

    iota_np = np.tile(np.arange(P, dtype=np.float32), (P, 1)).astype(
        ml_dtypes.bfloat16)
    ident_np = np.eye(P, dtype=np.float32).astype(ml_dtypes.bfloat16)
    # sentinel row: zero feats, al_src = -40 (f32 packed in bf16 slots 64..71)
    sent_view = np.zeros(P, np.uint16)
    sent_view[64:72] = np.full(4, SENT_ALS, np.float32).view(np.uint16)
    sent = sent_view.view(ml_dtypes.bfloat16)

    h = x
    for li in range(3):
        F = IN if li == 0 else 64
        l2 = li == 2
        Waug, skipWf, biasv, w2 = Ls[li]
        akey = ("A", F)
        if akey not in _CACHE:
            _CACHE[akey] = build_dense(F)
        bkey = ("B", l2)
        if bkey not in _CACHE:
            _CACHE[bkey] = build_edge(shared, l2)

        hT = h.T.astype(np.float32)
        base_a = {
            "Waug": _bf(Waug),
            "skipW": _bf(skipWf),
            "biasR": np.tile(np.asarray(biasv, np.float32), (P, 1)),
        }
        a_maps = []
        for c in range(NC):
            node = c * SB + PERMC
            valid_s = PERMC < SB
            hts = np.zeros((F, DP), np.float32)
            hts[:, valid_s] = hT[:, node[valid_s]]
            vm = plans[c]["vmap"][PERMC]
            valid_d = vm >= 0
            htow = np.zeros((F, DP), np.float32)
            htow[:, valid_d] = hT[:, vm[valid_d]]
            a_maps.append(dict(base_a, hTs=_bf(hts), hTow=_bf(htow)))
        res_a = run_bass_kernel_spmd(_CACHE[akey], a_maps,
                                     core_ids=list(range(NC)))
        _RESULTS.append(res_a)
        if res_a.exec_time_ns:
            _EXEC_NS.append(res_a.exec_time_ns)

        tbl = np.empty((TROWS, P), ml_dtypes.bfloat16)
        for c in range(4):
            tbl[c * RB:(c + 1) * RB] = res_a.results[c]["tshard"]
        tbl[LO_ROWS] = sent
        for c in range(4, 8):
            tbl[LO_ROWS + 1 + (c - 4) * RB: LO_ROWS + 1 + (c - 3) * RB] = \
                res_a.results[c]["tshard"]
        tbl[TROWS - 1] = sent

        base_b = {"table": tbl, "iota": iota_np}
        if l2:
            base_b["w2"] = _bf(w2)
            base_b["ident"] = ident_np
        b_maps = []
        for c in range(NC):
            aldv = res_a.results[c]["aldv"]
            alde = aldv[plans[c]["slot_dst"]]  # [128, nchunks, 4]
            b_maps.append(dict(
                base_b,
                idx_lo=plans[c]["idx_lo"], idx_hi=plans[c]["idx_hi"],
                dstloc=plans[c]["dstloc"],
                alde=np.ascontiguousarray(alde.astype(np.float32)),
                skipd=np.ascontiguousarray(
                    res_a.results[c]["skipd"].astype(np.float32)),
                featd=np.ascontiguousarray(res_a.results[c]["featd"]),
                selfz=np.ascontiguousarray(
                    res_a.results[c]["selfz"].astype(np.float32))))
        res_b = run_bass_kernel_spmd(_CACHE[bkey], b_maps,
                                     core_ids=list(range(NC)))
        _RESULTS.append(res_b)
        if res_b.exec_time_ns:
            _EXEC_NS.append(res_b.exec_time_ns)

        hn = np.zeros((N, 64), np.float32)
        for c in range(NC):
            vm = plans[c]["vmap"]
            valid = vm >= 0
            hn[vm[valid]] = res_b.results[c]["y"][valid]
        h = hn
        _DBG.append(h)
    return h


# revision 31
# speedup vs baseline: 4.3436x; 1.1355x over previous
"""GAT (3-layer, PyG-style) Trainium2 Bass kernel, 8-core dst-sharded. v2.

Self-contained: takes full inputs, shards internally, returns full output.

Design (per layer -> two SPMD launches):
  Launch A (dense, src-sharded): each core computes its 1/8 of the gather
    table rows [64 feat bf16 | 4 al_src f32] = 256B/row via PE matmuls with
    Waug = [W(*bns) | W@blk(a_src) | W@blk(a_dst)], plus per-dst al_dst and
    the dense skip+bias rows for its dst shard. Host assembles the full
    table (plus 2 sentinel rows) and expands the per-slot al_dst stream.
  Launch B (edge, dst-sharded): exact-CSR edge slots (chunks of 128 slots,
    one (window, src-half) per chunk; schedule shared across cores, per-core
    counts balanced by an LPT assignment of dst nodes to (core, window)).
    Per group of chunks: dma_gather of 256B rows (4 SWDGE queues round-
    robin), ex = exp(leaky_relu(al_s + al_d)) on DVE/Act, messages scaled
    by ex, segment-sum via per-chunk one-hot matmuls (sel built on DVE by
    comparing a shipped dstloc column against an iota row) accumulating
    [msg | sum_ex] per 128-dst window in PSUM, drained into an SBUF
    accumulator. One batched normalize pass at the end.
  Layer 2 uses GAT linearity: segment-sum runs in h1-space (same 256B rows)
    with 4 per-head ex-scaled copies; W2 is applied per window afterwards
    (PE transpose + 2 matmuls), then mean over heads.
"""
import numpy as np
import ml_dtypes

import concourse.bacc as bacc
import concourse.mybir as mybir
import concourse.tile as tile
from concourse.alu_op_type import AluOpType
from concourse.bass_utils import run_bass_kernel_spmd

BF16 = mybir.dt.bfloat16
F32 = mybir.dt.float32
I16 = mybir.dt.int16

NC = 8
P = 128
EPS = 1e-5
SENT_ALS = -40.0
NQ = 4          # SWDGE queues
GC01 = 64       # chunks per gather group, layers 0/1
GC2 = 40        # chunks per gather group, layer 2

N_NODES = 50000
SB = N_NODES // NC          # src nodes per core (6250)
NWIN = 51                   # dst windows per core
WCAP = 125                  # dsts per window (< 128 for ceil slack)
DP = NWIN * P               # padded dst slots per core (6528)
RB = DP                     # table row block per core (6272)
LO_ROWS = 4 * RB            # rows in the lo half (25088)
SENT_REL = LO_ROWS          # sentinel index relative to each half base
TROWS = 2 * (LO_ROWS + 1)


def _ceil(a, b):
    return -(-a // b)


# ----------------------------------------------------------------- planning

def build_plan(src, dst, N):
    assert N == N_NODES
    # src -> permuted table row (relative to half base) and half flag
    s_core = src // SB
    arow = (src % SB) + s_core * RB
    half = (s_core >= 4).astype(np.int64)
    rel = np.where(half == 0, arow, arow - 4 * RB)

    # balanced assignment of dst nodes to (core, window) buckets, keeping
    # BOTH halves' per-bucket loads even (ceil(max_c/128) drives slot count)
    deg = np.zeros((N, 2), np.int64)
    np.add.at(deg, (dst, half), 1)
    tot = deg.sum(1)
    order = np.argsort(-tot, kind="stable")
    NB = NC * NWIN
    LO = np.zeros(NB, np.float64)
    HI = np.zeros(NB, np.float64)
    bcount = np.zeros(NB, np.int64)
    BIG = 1e18
    dst_c = np.empty(N, np.int64)
    dst_w = np.empty(N, np.int64)
    dst_p = np.empty(N, np.int64)
    for n in order:
        score = np.maximum(LO + deg[n, 0], HI + deg[n, 1])
        b = int(np.argmin(score))
        dst_c[n] = b // NWIN
        dst_w[n] = b % NWIN
        dst_p[n] = bcount[b]
        bcount[b] += 1
        LO[b] += deg[n, 0]
        HI[b] += deg[n, 1]
        if bcount[b] >= WCAP:
            LO[b] = HI[b] = BIG

    vmap = np.full((NC, DP), -1, np.int64)
    vmap[dst_c, dst_w * P + dst_p] = np.arange(N)

    # per (core, window, half) edge counts -> shared chunk schedule
    ec = dst_c[dst]
    ew = dst_w[dst]
    epos = dst_p[dst]
    cnt = np.zeros((NC, NWIN, 2), np.int64)
    np.add.at(cnt, (ec, ew, half), 1)
    nch = np.zeros((NWIN, 2), np.int64)
    for w in range(NWIN):
        for h in range(2):
            nch[w, h] = _ceil(int(cnt[:, w, h].max()), P) if cnt[:, w, h].max() else 0
        if nch[w].sum() == 0:
            nch[w, 0] = 1  # keep every window on the schedule
    chunks = []          # (w, h) in schedule order: lo sweep then hi sweep
    for h in range(2):
        for w in range(NWIN):
            chunks += [(w, h)] * int(nch[w, h])
    nchunks = len(chunks)
    slot_base = {}       # (w, h) -> first slot index in the combined stream
    o = 0
    for (w, h) in chunks:
        slot_base.setdefault((w, h), o * P)
        o += 1
    nlo = int(nch[:, 0].sum())
    SLO, SHI = nlo * P, (nchunks - nlo) * P

    # per-core streams
    plans = []
    for c in range(NC):
        em = ec == c
        eh = half[em]
        key = eh * (NWIN * P) + ew[em] * P  # per (h, w) group key base
        # stable ordering by (h, w); position within group via argsort
        si = np.argsort(key, kind="stable")
        erel = rel[em][si]
        ewk = ew[em][si]
        ehk = eh[si]
        epk = epos[em][si]
        idx_all = np.full(nchunks * P, SENT_REL, np.int64)
        dloc = np.full(nchunks * P, -1.0, np.float32)
        sdst = np.zeros(nchunks * P, np.int64)
        # group runs: edges sorted by (h, w); fill each group's slot range
        bounds = np.searchsorted(
            ehk * NWIN + ewk, np.arange(2 * NWIN + 1))
        for h in range(2):
            for w in range(NWIN):
                a, b = bounds[h * NWIN + w], bounds[h * NWIN + w + 1]
                if a == b:
                    continue
                s0 = slot_base[(w, h)]
                m = b - a
                idx_all[s0: s0 + m] = erel[a:b]
                dloc[s0: s0 + m] = epk[a:b]
                sdst[s0: s0 + m] = w * P + epk[a:b]
        idx_lo = _wrap16(idx_all[:SLO])
        idx_hi = _wrap16(idx_all[SLO:])
        dstloc = dloc.reshape(nchunks, P).T.astype(ml_dtypes.bfloat16)
        sdst_w = sdst.reshape(nchunks, P).transpose(1, 0)
        plans.append(dict(vmap=vmap[c], idx_lo=idx_lo, idx_hi=idx_hi,
                          dstloc=np.ascontiguousarray(dstloc),
                          slot_dst=np.ascontiguousarray(sdst_w)))
    shared = dict(chunks=chunks, nchunks=nchunks, nlo=nlo, SLO=SLO, SHI=SHI)
    return shared, plans


def _wrap16(stream):
    S = len(stream)
    w = stream.reshape(S // 16, 16).T.astype(np.int16)
    return np.ascontiguousarray(np.tile(w, (8, 1)))


PERMC = (np.arange(DP) % P) * NWIN + np.arange(DP) // P  # col k -> slot p*49+i


# ------------------------------------------------------------- launch A (dense)

def build_dense(F):
    nc = bacc.Bacc("TRN2", target_bir_lowering=False, debug=False)
    hTs = nc.dram_tensor("hTs", [F, DP], BF16, kind="ExternalInput")
    hTow = nc.dram_tensor("hTow", [F, DP], BF16, kind="ExternalInput")
    Waug = nc.dram_tensor("Waug", [F, 72], BF16, kind="ExternalInput")
    skipW = nc.dram_tensor("skipW", [F, 64], BF16, kind="ExternalInput")
    biasR = nc.dram_tensor("biasR", [P, 64], F32, kind="ExternalInput")
    tshard = nc.dram_tensor("tshard", [DP, P], BF16, kind="ExternalOutput")
    aldv = nc.dram_tensor("aldv", [DP, 4], F32, kind="ExternalOutput")
    skipd = nc.dram_tensor("skipd", [DP, 64], F32, kind="ExternalOutput")
    featd = nc.dram_tensor("featd", [DP, 64], BF16, kind="ExternalOutput")
    selfz = nc.dram_tensor("selfz", [DP, 4], F32, kind="ExternalOutput")

    with tile.TileContext(nc) as tc:
        with (
            tc.tile_pool(name="c", bufs=1) as cp,
            tc.tile_pool(name="ps", bufs=2, space="PSUM") as pp,
        ):
            hts_sb = cp.tile([F, DP], BF16)
            nc.sync.dma_start(hts_sb[:], hTs[:])
            htow_sb = cp.tile([F, DP], BF16)
            nc.sync.dma_start(htow_sb[:], hTow[:])
            waug_sb = cp.tile([F, 72], BF16)
            nc.sync.dma_start(waug_sb[:], Waug[:])
            skipw_sb = cp.tile([F, 64], BF16)
            nc.sync.dma_start(skipw_sb[:], skipW[:])
            bias_sb = cp.tile([P, 64], F32)
            nc.sync.dma_start(bias_sb[:], biasR[:])

            tstage = cp.tile([P, NWIN, P], BF16)
            asb = cp.tile([P, NWIN, 4], F32)
            ssb = cp.tile([P, NWIN, 64], F32)
            fsb = cp.tile([P, NWIN, 64], BF16)
            szb = cp.tile([P, NWIN, 4], F32)
            nc.gpsimd.memset(tstage[:], 0)
            for i in range(NWIN):
                dps = pp.tile([P, 72], F32, space="PSUM", tag="dps")
                nc.tensor.matmul(dps[:], hts_sb[:, i * P:(i + 1) * P],
                                 waug_sb[:], start=True, stop=True)
                nc.vector.tensor_copy(tstage[:, i, 0:64], dps[:, 0:64])
                tf32 = tstage[:].bitcast(F32)
                nc.vector.tensor_copy(tf32[:, i, 32:36], dps[:, 64:68])
                ops_ = pp.tile([P, 72], F32, space="PSUM", tag="ops")
                nc.tensor.matmul(ops_[:], htow_sb[:, i * P:(i + 1) * P],
                                 waug_sb[:], start=True, stop=True)
                sps = pp.tile([P, 64], F32, space="PSUM", tag="sps")
                nc.tensor.matmul(sps[:], htow_sb[:, i * P:(i + 1) * P],
                                 skipw_sb[:], start=True, stop=True)
                nc.vector.tensor_copy(asb[:, i, :], ops_[:, 68:72])
                nc.vector.tensor_copy(fsb[:, i, :], ops_[:, 0:64])
                nc.vector.tensor_tensor(szb[:, i, :], ops_[:, 64:68],
                                        asb[:, i, :], AluOpType.add)
                nc.vector.tensor_tensor(ssb[:, i, :], sps[:],
                                        bias_sb[:], AluOpType.add)
            nc.sync.dma_start(
                tshard[:].rearrange("(p i) w -> p i w", p=P), tstage[:])
            nc.sync.dma_start(
                aldv[:].rearrange("(p i) h -> p i h", p=P), asb[:])
            nc.sync.dma_start(
                skipd[:].rearrange("(p i) c -> p i c", p=P), ssb[:])
            nc.sync.dma_start(
                featd[:].rearrange("(p i) c -> p i c", p=P), fsb[:])
            nc.sync.dma_start(
                selfz[:].rearrange("(p i) h -> p i h", p=P), szb[:])
    nc.compile()
    return nc


# ------------------------------------------------------------- launch B (edge)

def build_edge(shared, l2):
    chunks = shared["chunks"]
    nchunks = shared["nchunks"]
    nlo = shared["nlo"]
    SLO, SHI = shared["SLO"], shared["SHI"]
    GC = GC2 if l2 else GC01
    NW = 260 if l2 else 68

    nc = bacc.Bacc("TRN2", target_bir_lowering=False, debug=False,
                   num_swdge_queues=NQ)
    table = nc.dram_tensor("table", [TROWS, P], BF16, kind="ExternalInput")
    idx_lo = nc.dram_tensor("idx_lo", [P, max(SLO, 16) // 16], I16,
                            kind="ExternalInput")
    idx_hi = nc.dram_tensor("idx_hi", [P, max(SHI, 16) // 16], I16,
                            kind="ExternalInput")
    dstloc = nc.dram_tensor("dstloc", [P, nchunks], BF16,
                            kind="ExternalInput")
    alde_in = nc.dram_tensor("alde", [P, nchunks, 4], F32,
                             kind="ExternalInput")
    skipd_in = nc.dram_tensor("skipd", [DP, 64], F32, kind="ExternalInput")
    featd_in = nc.dram_tensor("featd", [DP, 64], BF16, kind="ExternalInput")
    selfz_in = nc.dram_tensor("selfz", [DP, 4], F32, kind="ExternalInput")
    iota_in = nc.dram_tensor("iota", [P, P], BF16, kind="ExternalInput")
    if l2:
        w2_in = nc.dram_tensor("w2", [P, 2, 64], BF16, kind="ExternalInput")
        ident_in = nc.dram_tensor("ident", [P, P], BF16, kind="ExternalInput")
    y_out = nc.dram_tensor("y", [DP, 64], F32, kind="ExternalOutput")

    # group schedule: runs of <= GC chunks, same half
    groups = []
    k = 0
    while k < nchunks:
        k1 = min(k + GC, nchunks, nlo if k < nlo else nchunks)
        groups.append((k, k1))
        k = k1
    first = [False] * nchunks
    last = [False] * nchunks
    wlast = [False] * nchunks
    seen = set()
    wl = {}
    for k, (w, h) in enumerate(chunks):
        if (h, w) not in seen:
            seen.add((h, w))
            first[k] = True
        if k + 1 >= nchunks or chunks[k + 1] != (w, h):
            last[k] = True
        wl[w] = k
    for w, k in wl.items():
        wlast[k] = True

    with tile.TileContext(nc) as tc:
        with (
            tc.tile_pool(name="c", bufs=1) as cp,
            tc.tile_pool(name="g", bufs=3 if l2 else 4) as gp,
            tc.tile_pool(name="r", bufs=2) as rp,
            tc.tile_pool(name="s", bufs=3) as sp,
            tc.tile_pool(name="pw", bufs=4, space="PSUM") as pw,
            tc.tile_pool(name="pt", bufs=2, space="PSUM") as pt,
        ):
            # big constant loads go on the Act engine's DGE so the first
            # gather groups' idx loads (sync engine) aren't queued behind them
            dloc_sb = cp.tile([P, nchunks], BF16)
            nc.scalar.dma_start(dloc_sb[:], dstloc[:])
            alde_sb = cp.tile([P, nchunks, 4], F32)
            nc.scalar.dma_start(alde_sb[:], alde_in[:])
            skipd_sb = cp.tile([P, NWIN, 64], F32)
            nc.scalar.dma_start(
                skipd_sb[:], skipd_in[:].rearrange("(i p) c -> p i c", p=P))
            featd_sb = cp.tile([P, NWIN, 64], BF16)
            nc.scalar.dma_start(
                featd_sb[:], featd_in[:].rearrange("(i p) c -> p i c", p=P))
            selfz_sb = cp.tile([P, NWIN, 4], F32)
            nc.scalar.dma_start(
                selfz_sb[:], selfz_in[:].rearrange("(i p) h -> p i h", p=P))
            iota_sb = cp.tile([P, 1, P], BF16)
            nc.scalar.dma_start(iota_sb[:, 0, :], iota_in[:])
            if l2:
                w2_sb = cp.tile([P, 2, 64], BF16)
                nc.scalar.dma_start(w2_sb[:], w2_in[:])
                ident_sb = cp.tile([P, P], BF16)
                nc.scalar.dma_start(ident_sb[:], ident_in[:])
            msum = cp.tile([P, NWIN, NW], F32)
            y_sb = cp.tile([P, NWIN, 64], F32)

            # init msum with the dense self-loop contributions
            exself = cp.tile([P, NWIN, 4], F32)
            nc.vector.scalar_tensor_tensor(
                exself[:], selfz_sb[:], 0.2, selfz_sb[:],
                AluOpType.mult, AluOpType.max)
            nc.scalar.activation(exself[:], exself[:],
                                 mybir.ActivationFunctionType.Exp)
            nc.vector.tensor_copy(msum[:, :, NW - 4: NW], exself[:])
            if l2:
                nc.vector.tensor_tensor(
                    msum[:, :, 0:256].rearrange("p w (h c) -> p w h c", c=64),
                    featd_sb[:, :, None, :].to_broadcast([P, NWIN, 4, 64]),
                    exself[:, :, :, None].to_broadcast([P, NWIN, 4, 64]),
                    AluOpType.mult)
            else:
                nc.vector.tensor_tensor(
                    msum[:, :, 0:64].rearrange("p w (h c) -> p w h c", c=16),
                    featd_sb[:].rearrange("p w (h c) -> p w h c", c=16),
                    exself[:, :, :, None].to_broadcast([P, NWIN, 4, 16]),
                    AluOpType.mult)

            win_ps = {}
            for gi, (k0, k1) in enumerate(groups):
                T = k1 - k0
                h = chunks[k0][1]
                base = table[0: LO_ROWS + 1, :] if h == 0 \
                    else table[LO_ROWS + 1: TROWS, :]
                o16 = (k0 * P if h == 0 else (k0 - nlo) * P) // 16
                idx_t = sp.tile([P, GC * 8], I16, tag="idx")
                nc.sync.dma_start(
                    idx_t[:, : T * 8],
                    (idx_lo if h == 0 else idx_hi)[:, o16: o16 + T * 8])
                gt = gp.tile([P, GC, P], BF16, tag="g")
                nc.gpsimd.dma_gather(
                    gt[:, :T, :], base, idx_t[:, : T * 8], T * P, T * P, P,
                    single_packet=False, queue_num=gi % NQ)

                zf = sp.tile([P, GC, 4], F32, tag="z")
                gf = gt[:].bitcast(F32)
                nc.vector.tensor_tensor(zf[:, :T, :], gf[:, :T, 32:36],
                                        alde_sb[:, k0:k1, :], AluOpType.add)
                nc.vector.scalar_tensor_tensor(
                    zf[:, :T, :], zf[:, :T, :], 0.2, zf[:, :T, :],
                    AluOpType.mult, AluOpType.max)
                sel = sp.tile([P, GC, P], BF16, tag="sel")
                nc.vector.tensor_tensor(
                    sel[:, :T, :],
                    iota_sb[:].to_broadcast([P, T, P]),
                    dloc_sb[:, k0:k1, None].to_broadcast([P, T, P]),
                    AluOpType.is_equal)

                if l2:
                    rhs = rp.tile([P, GC, 260], BF16, tag="rhs")
                    nc.scalar.activation(rhs[:, :T, 256:260], zf[:, :T, :],
                                         mybir.ActivationFunctionType.Exp)
                    nc.vector.tensor_tensor(
                        rhs[:, :T, 0:256].rearrange(
                            "p t (h c) -> p t h c", c=64),
                        gt[:, :T, None, 0:64].to_broadcast([P, T, 4, 64]),
                        rhs[:, :T, 256:260, None].to_broadcast([P, T, 4, 64]),
                        AluOpType.mult)
                else:
                    nc.scalar.activation(gt[:, :T, 64:68], zf[:, :T, :],
                                         mybir.ActivationFunctionType.Exp)
                    nc.vector.tensor_tensor(
                        gt[:, :T, 0:64].rearrange("p t (h c) -> p t h c", c=16),
                        gt[:, :T, 0:64].rearrange("p t (h c) -> p t h c", c=16),
                        gt[:, :T, 64:68, None].to_broadcast([P, T, 4, 16]),
                        AluOpType.mult)

                for t in range(T):
                    k = k0 + t
                    w, hh = chunks[k]
                    if first[k]:
                        win_ps[w] = pw.tile([P, NW], F32, space="PSUM",
                                            tag="win", name=f"win{w}h{hh}")
                    rhs_ap = rhs[:, t, :] if l2 else gt[:, t, 0:68]
                    nc.tensor.matmul(win_ps[w][:], sel[:, t, :], rhs_ap,
                                     start=first[k], stop=last[k],
                                     skip_group_check=True)
                    if last[k]:
                        pwin = win_ps.pop(w)
                        nc.vector.tensor_tensor(msum[:, w, :], msum[:, w, :],
                                                pwin[:], AluOpType.add)
                    if l2 and wlast[k]:
                        # per-window W2 drain, overlapped with later groups
                        recw = sp.tile([P, 4], F32, tag="recw")
                        nc.vector.reciprocal(recw[:], msum[:, w, 256:260])
                        snw = sp.tile([P, 4, 64], BF16, tag="snw")
                        nc.vector.tensor_tensor(
                            snw[:],
                            msum[:, w, 0:256].rearrange(
                                "p (h c) -> p h c", c=64),
                            recw[:, :, None].to_broadcast([P, 4, 64]),
                            AluOpType.mult)
                        yps = pt.tile([P, 64], F32, space="PSUM", tag="yps")
                        for j in range(2):
                            tp = pt.tile([P, P], BF16, space="PSUM", tag="tp")
                            nc.tensor.matmul(
                                tp[:],
                                snw[:].rearrange("p h c -> p (h c)")
                                      [:, j * P:(j + 1) * P],
                                ident_sb[:], is_transpose=True,
                                start=True, stop=True, skip_group_check=True)
                            st = sp.tile([P, P], BF16, tag="st")
                            nc.scalar.activation(
                                st[:], tp[:],
                                mybir.ActivationFunctionType.Copy)
                            nc.tensor.matmul(yps[:], st[:], w2_sb[:, j, :],
                                             start=(j == 0), stop=(j == 1),
                                             skip_group_check=True)
                        nc.vector.scalar_tensor_tensor(
                            y_sb[:, w, :], yps[:], 0.25, skipd_sb[:, w, :],
                            AluOpType.mult, AluOpType.add)

            if not l2:
                rec = cp.tile([P, NWIN, 4], F32)
                nc.vector.reciprocal(rec[:], msum[:, :, 64:68])
                nc.vector.tensor_tensor(
                    y_sb[:].rearrange("p w (h c) -> p w h c", c=16),
                    msum[:, :, 0:64].rearrange("p w (h c) -> p w h c", c=16),
                    rec[:, :, :, None].to_broadcast([P, NWIN, 4, 16]),
                    AluOpType.mult)
                nc.vector.tensor_tensor(y_sb[:], y_sb[:], skipd_sb[:],
                                        AluOpType.add)
                nc.vector.tensor_scalar_max(y_sb[:], y_sb[:], 0.0)
            nc.sync.dma_start(
                y_out[:].rearrange("(i p) c -> p i c", p=P), y_sb[:])
    nc.compile()
    return nc


# ------------------------------------------------------------------ driver

_CACHE = {}
_DBG = []
_EXEC_NS = []
_RESULTS = []


def _blockdiag(a):
    H, C = a.shape
    m = np.zeros((H * C, H), np.float32)
    for hh in range(H):
        m[hh * C: (hh + 1) * C, hh] = a[hh]
    return m


def _bf(x):
    return np.ascontiguousarray(np.asarray(x, np.float32)
                                .astype(ml_dtypes.bfloat16))


def kernel(**inp):
    x = np.asarray(inp["x"], np.float32)
    ei = np.asarray(inp["edge_index"], np.int64)
    N, IN = x.shape
    E = ei.shape[1]

    # self-loops are handled densely in launch B; streams carry real edges
    src = ei[0]
    dst = ei[1]

    pkey = ("plan", N, E, hash(ei.tobytes()))
    if pkey not in _CACHE:
        _CACHE[pkey] = build_plan(src, dst, N)
    shared, plans = _CACHE[pkey]

    def prep01(Wv, a_s, a_d, cb, sW, sb, g, b, m, v):
        Wv, sW = np.asarray(Wv, np.float32), np.asarray(sW, np.float32)
        bns = (np.asarray(g) / np.sqrt(np.asarray(v) + EPS)).astype(np.float32)
        bnt = (np.asarray(b) - np.asarray(m) * bns).astype(np.float32)
        Waug = np.concatenate(
            [Wv * bns[None, :], Wv @ _blockdiag(np.asarray(a_s)),
             Wv @ _blockdiag(np.asarray(a_d))], 1)
        return (Waug, sW * bns[None, :],
                (np.asarray(cb) + np.asarray(sb)) * bns + bnt, None)

    def prep2(Wv, a_s, a_d, cb, sW, sb):
        Wv = np.asarray(Wv, np.float32)
        Waug = np.concatenate(
            [np.eye(64, dtype=np.float32), Wv @ _blockdiag(np.asarray(a_s)),
             Wv @ _blockdiag(np.asarray(a_d))], 1)
        w2 = np.ascontiguousarray(
            Wv.reshape(64, 4, 64).transpose(1, 0, 2).reshape(256, 64)
            .reshape(2, 128, 64).transpose(1, 0, 2))
        return (Waug, np.asarray(sW, np.float32),
                np.asarray(cb) + np.asarray(sb), w2)

    Ls = [
        prep01(inp["conv0_W"], inp["conv0_as"], inp["conv0_ad"],
               inp["conv0_b"], inp["skip0_W"], inp["skip0_b"],
               inp["bn0_g"], inp["bn0_b"], inp["bn0_m"], inp["bn0_v"]),
        prep01(inp["conv1_W"], inp["conv1_as"], inp["conv1_ad"],
               inp["conv1_b"], inp["skip1_W"], inp["skip1_b"],
               inp["bn1_g"], inp["bn1_b"], inp["bn1_m"], inp["bn1_v"]),
        prep2(inp["conv2_W"], inp["conv2_as"], inp["conv2_ad"],
              inp["conv2_b"], inp["skip2_W"], inp["skip2_b"]),
    ]

    iota_np = np.tile(np.arange(P, dtype=np.float32), (P, 1)).astype(
        ml_dtypes.bfloat16)
    ident_np = np.eye(P, dtype=np.float32).astype(ml_dtypes.bfloat16)
    # sentinel row: zero feats, al_src = -40 (f32 packed in bf16 slots 64..71)
    sent_view = np.zeros(P, np.uint16)
    sent_view[64:72] = np.full(4, SENT_ALS, np.float32).view(np.uint16)
    sent = sent_view.view(ml_dtypes.bfloat16)

    h = x
    for li in range(3):
        F = IN if li == 0 else 64
        l2 = li == 2
        Waug, skipWf, biasv, w2 = Ls[li]
        akey = ("A", F)
        if akey not in _CACHE:
            _CACHE[akey] = build_dense(F)
        bkey = ("B", l2)
        if bkey not in _CACHE:
            _CACHE[bkey] = build_edge(shared, l2)

        hT = h.T.astype(np.float32)
        base_a = {
            "Waug": _bf(Waug),
            "skipW": _bf(skipWf),
            "biasR": np.tile(np.asarray(biasv, np.float32), (P, 1)),
        }
        a_maps = []
        for c in range(NC):
            node = c * SB + PERMC
            valid_s = PERMC < SB
            hts = np.zeros((F, DP), np.float32)
            hts[:, valid_s] = hT[:, node[valid_s]]
            vm = plans[c]["vmap"][PERMC]
            valid_d = vm >= 0
            htow = np.zeros((F, DP), np.float32)
            htow[:, valid_d] = hT[:, vm[valid_d]]
            a_maps.append(dict(base_a, hTs=_bf(hts), hTow=_bf(htow)))
        res_a = run_bass_kernel_spmd(_CACHE[akey], a_maps,
                                     core_ids=list(range(NC)))
        _RESULTS.append(res_a)
        if res_a.exec_time_ns:
            _EXEC_NS.append(res_a.exec_time_ns)

        tbl = np.empty((TROWS, P), ml_dtypes.bfloat16)
        for c in range(4):
            tbl[c * RB:(c + 1) * RB] = res_a.results[c]["tshard"]
        tbl[LO_ROWS] = sent
        for c in range(4, 8):
            tbl[LO_ROWS + 1 + (c - 4) * RB: LO_ROWS + 1 + (c - 3) * RB] = \
                res_a.results[c]["tshard"]
        tbl[TROWS - 1] = sent

        base_b = {"table": tbl, "iota": iota_np}
        if l2:
            base_b["w2"] = _bf(w2)
            base_b["ident"] = ident_np
        b_maps = []
        for c in range(NC):
            aldv = res_a.results[c]["aldv"]
            alde = aldv[plans[c]["slot_dst"]]  # [128, nchunks, 4]
            b_maps.append(dict(
                base_b,
                idx_lo=plans[c]["idx_lo"], idx_hi=plans[c]["idx_hi"],
                dstloc=plans[c]["dstloc"],
                alde=np.ascontiguousarray(alde.astype(np.float32)),
                skipd=np.ascontiguousarray(
                    res_a.results[c]["skipd"].astype(np.float32)),
                featd=np.ascontiguousarray(res_a.results[c]["featd"]),
                selfz=np.ascontiguousarray(
                    res_a.results[c]["selfz"].astype(np.float32))))
        res_b = run_bass_kernel_spmd(_CACHE[bkey], b_maps,
                                     core_ids=list(range(NC)))
        _RESULTS.append(res_b)
        if res_b.exec_time_ns:
            _EXEC_NS.append(res_b.exec_time_ns)

        hn = np.zeros((N, 64), np.float32)
        for c in range(NC):
            vm = plans[c]["vmap"]
            valid = vm >= 0
            hn[vm[valid]] = res_b.results[c]["y"][valid]
        h = hn
        _DBG.append(h)
    return h
